# revision 17
# baseline (speedup 1.0000x reference)
"""Trainium2 Bass kernel for a dense transformer encoder layer.

Model (faithful to the oracle):
  q,k,v = x@wq+bq, x@wk+bk, x@wv+bv          (12 heads, dk=64, DIM=768)
  scores = q@k^T / sqrt(768)  (note: sqrt(dim_model), not sqrt(dk))
  scores[mask==0] = 1e-11  (NOT -inf; masked keys still contribute ~1/Z)
  attn = softmax(scores); z = attn@v; o = z@wo+bo
  l1 = x + LN(o);  ffn = relu(l1@w1+b1)@w2+b2;  out = l1 + LN(ffn)

Sharding: 4096 tokens (B=2,S=2048) split 8 ways -> 512 tokens/core.
Cores 0-3 own batch 0, cores 4-7 batch 1. K/V are computed for the
core's whole batch (redundantly within each 4-core group): measured
faster than all-gathering them (no collective barrier/trigger stalls,
and the extra matmuls keep the PE array HAM-warm).

Masking trick: mask*1/sqrt(768) is folded into K^T at the K-projection
bias-add (scalar_tensor_tensor: (k+bk)*msc), so masked key columns are
exactly 0 -> score 0 -> exp(0)=1.0 == fp32(exp(1e-11)). This makes the
exp scale-free, so it is batched 3 score-tiles per ACTIVATE (amortizes
the ~350-cycle ScalarE instruction overhead).

Softmax denominator comes from a ones column appended to V (attn@v
with M=65); normalization happens after attn@v via a rank-1 matmul
broadcast of 1/sum.
"""

import math
import os
import sys

import numpy as np

for _p in ("/opt/trn_rl_repo", os.path.expanduser("~/.axon_site/_ro/trn_rl_repo")):
    if os.path.isdir(_p) and _p not in sys.path:
        sys.path.insert(0, _p)

import ml_dtypes  # noqa: E402

BF16 = ml_dtypes.bfloat16

DIM = 768
HEADS = 12
DK = 64
HID = 4 * DIM  # 3072
B, S = 2, 2048
N_CORES = 8
BLK = 512            # tokens per core
NBLK = S // BLK      # 4 blocks per batch
EPS = 1e-5
ISCALE = 1.0 / math.sqrt(DIM)

FT = DIM // 128   # 6 feature tiles
TT = BLK // 128   # 4 token tiles per core block
ST = S // 128     # 16 key tiles per batch
HT = HID // 128   # 24 hidden tiles

# exp batching: groups of score k-tiles fused into one ACTIVATE
EXP_GROUPS = [(0, 3), (3, 6), (6, 9), (9, 12), (12, 15), (15, 16)]
EG = 3  # max group width (PSUM: 3 banks * 2 bufs + z + rb = 8 banks)

_CACHE: dict = {}
MAX_PHASE = int(os.environ.get("BASS_KERNEL_PHASES", "5"))


def _build_program():
    import concourse.bass as bass
    import concourse.mybir as mybir
    import concourse.tile as tile
    from concourse import bacc
    from concourse.masks import make_identity

    f32 = mybir.dt.float32
    bf16 = mybir.dt.bfloat16
    AF = mybir.ActivationFunctionType
    ALU = mybir.AluOpType
    AX = mybir.AxisListType

    nc = bacc.Bacc()

    # ---- per-core DRAM I/O ----
    d_xT = nc.dram_tensor("xT", [DIM, S], bf16, kind="ExternalInput")
    d_xTb = nc.dram_tensor("xTb", [DIM, BLK], bf16, kind="ExternalInput")
    d_xb = nc.dram_tensor("xb", [BLK, DIM], f32, kind="ExternalInput")
    d_msc = nc.dram_tensor("msc", [S], bf16, kind="ExternalInput")
    d_wq = nc.dram_tensor("wq", [DIM, DIM], bf16, kind="ExternalInput")
    d_wk = nc.dram_tensor("wk", [DIM, DIM], bf16, kind="ExternalInput")
    d_wv = nc.dram_tensor("wv", [DIM, DIM], bf16, kind="ExternalInput")
    d_wo = nc.dram_tensor("wo", [DIM, DIM], bf16, kind="ExternalInput")
    d_w1 = nc.dram_tensor("w1", [DIM, HID], bf16, kind="ExternalInput")
    d_w2 = nc.dram_tensor("w2", [HID, DIM], bf16, kind="ExternalInput")
    d_bq = nc.dram_tensor("bq", [DIM], f32, kind="ExternalInput")
    d_bk = nc.dram_tensor("bk", [DIM], f32, kind="ExternalInput")
    d_bv = nc.dram_tensor("bv", [DIM], f32, kind="ExternalInput")
    d_bo = nc.dram_tensor("bo", [DIM], f32, kind="ExternalInput")
    d_b1 = nc.dram_tensor("b1", [HID], f32, kind="ExternalInput")
    d_b2 = nc.dram_tensor("b2", [DIM], f32, kind="ExternalInput")
    d_g1 = nc.dram_tensor("g1", [DIM], f32, kind="ExternalInput")
    d_bb1 = nc.dram_tensor("bb1", [DIM], f32, kind="ExternalInput")
    d_g2 = nc.dram_tensor("g2", [DIM], f32, kind="ExternalInput")
    d_bb2 = nc.dram_tensor("bb2", [DIM], f32, kind="ExternalInput")
    d_out = nc.dram_tensor("out", [BLK, DIM], f32, kind="ExternalOutput")

    def bcast_ap(handle, n=128):
        ap = handle[:]
        return bass.AP(tensor=ap.tensor, offset=ap.offset, ap=[[0, n]] + list(ap.ap))

    with tile.TileContext(nc) as tc:
        with (
            tc.tile_pool(name="const", bufs=1) as const,
            tc.tile_pool(name="bigres", bufs=1) as big,
        ):
            # ---------- constants (small, off the critical DMA path) ----------
            sb_bk = const.tile([128, FT], f32)
            nc.sync.dma_start(out=sb_bk, in_=d_bk[:].rearrange("(t p) -> p t", p=128))
            sb_bq = const.tile([128, FT], f32)
            nc.sync.dma_start(out=sb_bq, in_=d_bq[:].rearrange("(t p) -> p t", p=128))
            sb_b1 = const.tile([128, HT], f32)
            nc.sync.dma_start(out=sb_b1, in_=d_b1[:].rearrange("(t p) -> p t", p=128))
            bo_bc = const.tile([128, DIM], f32)
            nc.gpsimd.dma_start(out=bo_bc, in_=bcast_ap(d_bo))
            b2_bc = const.tile([128, DIM], f32)
            nc.gpsimd.dma_start(out=b2_bc, in_=bcast_ap(d_b2))
            g1_bc = const.tile([128, DIM], f32)
            nc.gpsimd.dma_start(out=g1_bc, in_=bcast_ap(d_g1))
            bb1_bc = const.tile([128, DIM], f32)
            nc.gpsimd.dma_start(out=bb1_bc, in_=bcast_ap(d_bb1))
            g2_bc = const.tile([128, DIM], f32)
            nc.gpsimd.dma_start(out=g2_bc, in_=bcast_ap(d_g2))
            bb2_bc = const.tile([128, DIM], f32)
            nc.gpsimd.dma_start(out=bb2_bc, in_=bcast_ap(d_bb2))
            ident = const.tile([128, 128], f32)
            make_identity(nc, ident[:])
            ones64 = const.tile([1, 64], f32)
            nc.vector.memset(ones64, 1.0)
            eps_t = const.tile([128, 1], f32)
            nc.vector.memset(eps_t, EPS)

            # ---------- persistent activations ----------
            sb_xblk = big.tile([128, TT, DIM], f32)  # residual x (needed ph3)
            sb_l1 = big.tile([128, TT, DIM], f32)
            sb_zT = big.tile([128, FT, BLK], bf16)  # z^T normalized (ph2 -> ph3)

            # attention-scoped residents (freed before phase 3's LN pools)
            attn_res_cm = tc.tile_pool(name="attn_res", bufs=1)
            attn_res = attn_res_cm.__enter__()
            sb_K = attn_res.tile([128, FT, S], bf16)   # K^T*msc, feat-major
            sb_Q = attn_res.tile([128, FT, BLK], bf16)  # Q^T, feat-major
            sb_V = attn_res.tile([128, ST, HEADS, DK + 1], bf16)  # V + ones col

            # ============ Phase 1: QKV projections ============
            with (
                tc.tile_pool(name="xw", bufs=1) as xw,
                tc.tile_pool(name="ps1", bufs=4, space="PSUM") as ps1,
                tc.tile_pool(name="ps1v", bufs=4, space="PSUM") as ps1v,
            ):
                # phase-1-only broadcast constants
                msc_bc = xw.tile([128, S], bf16)
                nc.gpsimd.dma_start(out=msc_bc, in_=bcast_ap(d_msc))
                bv_bc = xw.tile([128, DIM], f32)
                nc.gpsimd.dma_start(out=bv_bc, in_=bcast_ap(d_bv))
                # critical-path order: Q's inputs (smallest) first, then K's
                # with xT split in halves so K matmuls start early
                w_q = xw.tile([128, FT, DIM], bf16)
                nc.sync.dma_start(
                    out=w_q, in_=d_wq[:].rearrange("(t p) o -> p t o", p=128)
                )
                sb_xTb = xw.tile([128, FT, BLK], bf16)
                nc.sync.dma_start(
                    out=sb_xTb, in_=d_xTb[:].rearrange("(t p) n -> p t n", p=128)
                )
                w_k = xw.tile([128, FT, DIM], bf16)
                nc.sync.dma_start(
                    out=w_k, in_=d_wk[:].rearrange("(t p) o -> p t o", p=128)
                )
                sb_xT = xw.tile([128, FT, S], bf16)
                nc.sync.dma_start(
                    out=sb_xT[:, :, 0 : S // 2],
                    in_=d_xT[:, 0 : S // 2].rearrange("(t p) n -> p t n", p=128),
                )
                nc.sync.dma_start(
                    out=sb_xT[:, :, S // 2 : S],
                    in_=d_xT[:, S // 2 : S].rearrange("(t p) n -> p t n", p=128),
                )
                w_v = xw.tile([128, FT, DIM], bf16)
                nc.sync.dma_start(
                    out=w_v, in_=d_wv[:].rearrange("(t p) o -> p t o", p=128)
                )
                # residual x: not needed until phase 3, queue it after
                nc.sync.dma_start(
                    out=sb_xblk, in_=d_xb[:].rearrange("(t p) d -> p t d", p=128)
                )

                # Q^T feat-major for the core's block (first DMAs to land)
                for ft in range(FT):
                    ps = ps1.tile([128, 512], f32, tag="p")
                    for kt in range(FT):
                        nc.tensor.matmul(
                            ps,
                            w_q[:, kt, ft * 128 : (ft + 1) * 128],
                            sb_xTb[:, kt, :],
                            start=(kt == 0),
                            stop=(kt == FT - 1),
                        )
                    nc.vector.tensor_scalar_add(
                        sb_Q[:, ft, :], ps, sb_bq[:, ft : ft + 1]
                    )
                # K^T feat-major over the whole batch, (K+bk)*msc fused;
                # nt-outer so the first xT half is enough to start
                for nt in range(S // 512):
                    for ft in range(FT):
                        ps = ps1.tile([128, 512], f32, tag="p")
                        for kt in range(FT):
                            nc.tensor.matmul(
                                ps,
                                w_k[:, kt, ft * 128 : (ft + 1) * 128],
                                sb_xT[:, kt, nt * 512 : (nt + 1) * 512],
                                start=(kt == 0),
                                stop=(kt == FT - 1),
                            )
                        nc.vector.scalar_tensor_tensor(
                            out=sb_K[:, ft, nt * 512 : (nt + 1) * 512],
                            in0=ps,
                            scalar=sb_bk[:, ft : ft + 1],
                            in1=msc_bc[:, nt * 512 : (nt + 1) * 512],
                            op0=ALU.add,
                            op1=ALU.mult,
                        )
                # V tok-major over the whole batch, laid out [tok, head, dk+1]
                nc.vector.memset(sb_V[:, :, :, DK : DK + 1], 1.0)
                for nh in range(2):
                    for tt in range(ST):
                        ps = ps1v.tile([128, 384], f32, tag="vp")
                        for kt in range(FT):
                            nc.tensor.matmul(
                                ps,
                                sb_xT[:, kt, tt * 128 : (tt + 1) * 128],
                                w_v[:, kt, nh * 384 : (nh + 1) * 384],
                                start=(kt == 0),
                                stop=(kt == FT - 1),
                            )
                        nc.vector.scalar_tensor_tensor(
                            out=sb_V[:, tt, nh * 6 : (nh + 1) * 6, 0:DK],
                            in0=ps[:].rearrange("p (h d) -> p h d", d=DK),
                            scalar=1.0,
                            in1=bv_bc[:, nh * 384 : (nh + 1) * 384].rearrange(
                                "p (h d) -> p h d", d=DK
                            ),
                            op0=ALU.mult,
                            op1=ALU.add,
                        )

            # prefetch wo now -- the DMA overlaps the attention phase
            w_o = big.tile([128, FT, DIM], bf16)
            nc.sync.dma_start(out=w_o, in_=d_wo[:].rearrange("(t p) o -> p t o", p=128))

            if MAX_PHASE >= 2:
                # ============ Phase 2: attention ============
                # PSUM budget: ps_sc 3 banks x 2 bufs + ps_z 1 x 2 bufs = 8.
                # The rank-1 1/Z broadcast borrows a ps_sc buffer briefly.
                with (
                    tc.tile_pool(name="expp", bufs=16) as expp,
                    tc.tile_pool(name="attsm", bufs=3) as attsm,
                    tc.tile_pool(name="ps_sc", bufs=2, space="PSUM") as ps_sc,
                    tc.tile_pool(name="ps_z", bufs=2, space="PSUM") as ps_z,
                ):
                    def z_chunk(zps, ets, hp, half, a, b):
                        h = 2 * hp + half
                        for kt2 in range(a, b):
                            nc.tensor.matmul(
                                zps[half],
                                sb_V[:, kt2, h, :],
                                ets[half][kt2],
                                start=(kt2 == 0),
                                stop=(kt2 == ST - 1),
                            )

                    def z_tail(zps, hp, half):
                        ho = half * 64
                        zp = zps[half]
                        rsum = attsm.tile([1, BLK], f32, tag="rsum")
                        nc.vector.reciprocal(rsum, zp[DK : DK + 1, :])
                        rbt = ps_sc.tile([128, EG * 512], f32, tag="sc")
                        rbp = rbt[0:64, 0:BLK]
                        nc.tensor.matmul(rbp, ones64[:], rsum, start=True, stop=True)
                        rb = attsm.tile([64, BLK], f32, tag="rbs")
                        nc.vector.tensor_copy(rb, rbp)
                        nc.vector.tensor_mul(
                            sb_zT[ho : ho + 64, ht, :], zp[0:DK, :], rb
                        )

                    for hp in range(HEADS // 2):
                        ht = hp
                        # The two heads of a pair use disjoint PE row groups
                        # (partitions 0-63 / 64-127): their K=64 score matmuls
                        # run concurrently in the array. z-matmul chunks are
                        # staggered one exp-group behind the score matmuls so
                        # the PE always has runnable work while ScalarE exps.
                        ets = ([], [])
                        zps = [
                            ps_z.tile([DK + 1, BLK], f32, tag="z", name="zp0"),
                            ps_z.tile([DK + 1, BLK], f32, tag="z", name="zp1"),
                        ]
                        for gi, (a, b) in enumerate(EXP_GROUPS):
                            g = b - a
                            for half in (0, 1):
                                ho = half * 64
                                ps = ps_sc.tile([128, EG * 512], f32, tag="sc")
                                for j, kt2 in enumerate(range(a, b)):
                                    nc.tensor.matmul(
                                        ps[:, j * 512 : (j + 1) * 512],
                                        sb_K[ho : ho + 64, ht, kt2 * 128 : (kt2 + 1) * 128],
                                        sb_Q[ho : ho + 64, ht, :],
                                        start=True,
                                        stop=True,
                                    )
                                et = expp.tile([128, EG * 512], bf16, tag="exp")
                                nc.scalar.activation(
                                    et[:, : g * 512], ps[:, : g * 512], AF.Exp
                                )
                                for j in range(g):
                                    ets[half].append(et[:, j * 512 : (j + 1) * 512])
                            if gi >= 1:
                                pa, pb = EXP_GROUPS[gi - 1]
                                for half in (0, 1):
                                    z_chunk(zps, ets, hp, half, pa, pb)
                        la, lb = EXP_GROUPS[-1]
                        for half in (0, 1):
                            z_chunk(zps, ets, hp, half, la, lb)
                            z_tail(zps, hp, half)

            attn_res_cm.__exit__(None, None, None)

            if MAX_PHASE >= 3:
                # ============ Phase 3: O proj + LN1 (+residual) ============
                def layer_norm_to(out_ap, x_ap, g_bc_t, resid_ap, pool):
                    s = pool.tile([128, 1], f32, tag="ln_s")
                    nc.vector.tensor_reduce(s, x_ap, axis=AX.X, op=ALU.add)
                    mean = pool.tile([128, 1], f32, tag="ln_m")
                    nc.scalar.mul(mean, s, 1.0 / DIM)
                    xc = pool.tile([128, DIM], f32, tag="ln_xc")
                    nc.vector.tensor_scalar(xc, x_ap, mean, None, op0=ALU.subtract)
                    junk = pool.tile([128, DIM], f32, tag="ln_j")
                    var = pool.tile([128, 1], f32, tag="ln_v")
                    # (tensor_tensor_reduce crashes the device on this runtime;
                    # scalar_tensor_tensor with accum_out works)
                    nc.vector.scalar_tensor_tensor(
                        out=junk, in0=xc, scalar=1.0, in1=xc,
                        op0=ALU.mult, op1=ALU.mult, accum_out=var,
                    )
                    nc.vector.tensor_scalar_mul(var, var, 1.0 / DIM)
                    sd = pool.tile([128, 1], f32, tag="ln_sd")
                    nc.scalar.activation(sd, var, AF.Sqrt, bias=eps_t[:])
                    rstd = pool.tile([128, 1], f32, tag="ln_r")
                    nc.vector.reciprocal(rstd, sd)
                    t = pool.tile([128, DIM], f32, tag="ln_t")
                    nc.vector.tensor_scalar(t, xc, rstd, None, op0=ALU.mult)
                    tg = pool.tile([128, DIM], f32, tag="ln_tg")
                    nc.vector.tensor_mul(tg, t, g_bc_t)
                    nc.vector.tensor_add(out_ap, tg, resid_ap)

                ffnp_cm = tc.tile_pool(name="ffnp", bufs=1)
                ffnp = ffnp_cm.__enter__()
                # w1 first (feeds FFN1 soon), w2 lands during FFN1; both
                # DMAs overlap phase 3's O-projection + LayerNorm
                w1_t = ffnp.tile([128, FT, HID], bf16)
                nc.sync.dma_start(
                    out=w1_t, in_=d_w1[:].rearrange("(t p) h -> p t h", p=128)
                )
                w2_t = ffnp.tile([128, HT, DIM], bf16)
                nc.sync.dma_start(
                    out=w2_t, in_=d_w2[:].rearrange("(t p) o -> p t o", p=128)
                )
                sb_hT = ffnp.tile([128, HT, BLK], bf16)  # relu(ffn1)^T
                sb_l1T = ffnp.tile([128, FT, BLK], bf16)

                with (
                    tc.tile_pool(name="ln1p", bufs=2) as ln1p,
                    tc.tile_pool(name="ps_o", bufs=4, space="PSUM") as ps_o,
                    tc.tile_pool(name="ps_t1", bufs=2, space="PSUM") as ps_t1,
                ):
                    for tt in range(TT):
                        l1pre = ln1p.tile([128, DIM], f32, tag="l1pre")
                        for nh in range(2):
                            ps = ps_o.tile([128, 384], f32, tag="op")
                            for kt in range(FT):
                                nc.tensor.matmul(
                                    ps,
                                    sb_zT[:, kt, tt * 128 : (tt + 1) * 128],
                                    w_o[:, kt, nh * 384 : (nh + 1) * 384],
                                    start=(kt == 0),
                                    stop=(kt == FT - 1),
                                )
                            nc.vector.scalar_tensor_tensor(
                                out=l1pre[:, nh * 384 : (nh + 1) * 384],
                                in0=ps,
                                scalar=1.0,
                                in1=bo_bc[:, nh * 384 : (nh + 1) * 384],
                                op0=ALU.mult,
                                op1=ALU.add,
                            )
                        xb1 = ln1p.tile([128, DIM], f32, tag="xb1")
                        nc.vector.tensor_add(xb1, sb_xblk[:, tt, :], bb1_bc)
                        layer_norm_to(sb_l1[:, tt, :], l1pre[:], g1_bc, xb1, ln1p)
                        # transpose this l1 tile immediately (feeds FFN1)
                        for ft in range(FT):
                            pst = ps_t1.tile([128, 128], f32, tag="tp")
                            nc.tensor.transpose(
                                pst, sb_l1[:, tt, ft * 128 : (ft + 1) * 128], ident[:]
                            )
                            nc.scalar.copy(
                                sb_l1T[:, ft, tt * 128 : (tt + 1) * 128], pst
                            )

            if MAX_PHASE >= 4:
                # ============ Phase 4+5: FFN (weights already resident) ====
                with (
                    tc.tile_pool(name="ln2p", bufs=2) as ln2p,
                    tc.tile_pool(name="outp", bufs=3) as outp,
                    tc.tile_pool(name="ps_f1", bufs=4, space="PSUM") as ps_f1,
                ):
                    for ht2 in range(HT):
                        ps = ps_f1.tile([128, BLK], f32, tag="f1")
                        for kt in range(FT):
                            nc.tensor.matmul(
                                ps,
                                w1_t[:, kt, ht2 * 128 : (ht2 + 1) * 128],
                                sb_l1T[:, kt, :],
                                start=(kt == 0),
                                stop=(kt == FT - 1),
                            )
                        # relu(x + b1) on DVE: (x add b1) max 0
                        nc.vector.tensor_scalar(
                            sb_hT[:, ht2, :], ps, sb_b1[:, ht2 : ht2 + 1], 0.0,
                            op0=ALU.add, op1=ALU.max,
                        )

                    if MAX_PHASE >= 5:
                        # ============ Phase 5: FFN2 + LN2 + out ============
                        out_r = d_out[:].rearrange("(t p) d -> p t d", p=128)
                        for tt in range(TT):
                            f2pre = ln2p.tile([128, DIM], f32, tag="f2pre")
                            for nh in range(2):
                                ps = ps_f1.tile([128, 384], f32, tag="f2")
                                for kt in range(HT):
                                    nc.tensor.matmul(
                                        ps,
                                        sb_hT[:, kt, tt * 128 : (tt + 1) * 128],
                                        w2_t[:, kt, nh * 384 : (nh + 1) * 384],
                                        start=(kt == 0),
                                        stop=(kt == HT - 1),
                                    )
                                nc.vector.scalar_tensor_tensor(
                                    out=f2pre[:, nh * 384 : (nh + 1) * 384],
                                    in0=ps,
                                    scalar=1.0,
                                    in1=b2_bc[:, nh * 384 : (nh + 1) * 384],
                                    op0=ALU.mult,
                                    op1=ALU.add,
                                )
                            l1b = ln2p.tile([128, DIM], f32, tag="l1b")
                            nc.vector.tensor_add(l1b, sb_l1[:, tt, :], bb2_bc)
                            o_sb = outp.tile([128, DIM], f32, tag="osb")
                            layer_norm_to(o_sb[:], f2pre[:], g2_bc, l1b, ln2p)
                            nc.sync.dma_start(out=out_r[:, tt, :], in_=o_sb)

            if MAX_PHASE >= 3:
                ffnp_cm.__exit__(None, None, None)

    return nc


def _get_nc(finalized=True):
    if "nc" not in _CACHE:
        _CACHE["nc"] = _build_program()
    nc = _CACHE["nc"]
    if finalized and not nc.is_finalized():
        nc.finalize()
    return nc


def make_in_maps(inputs: dict) -> list:
    x = np.asarray(inputs["x_n"], np.float32).reshape(B, S, DIM)
    mask = np.asarray(inputs["mask"]).reshape(B, S)
    w = {
        k: np.ascontiguousarray(np.asarray(inputs[k], np.float32).astype(BF16))
        for k in ("wq", "wk", "wv", "wo", "w1", "w2")
    }
    vecs = {
        "bq": inputs["bq"], "bk": inputs["bk"], "bv": inputs["bv"],
        "bo": inputs["bo"], "b1": inputs["b1"], "b2": inputs["b2"],
        "g1": inputs["ln1_g"], "bb1": inputs["ln1_b"],
        "g2": inputs["ln2_g"], "bb2": inputs["ln2_b"],
    }
    vecs = {k: np.ascontiguousarray(np.asarray(v, np.float32)) for k, v in vecs.items()}
    in_maps = []
    for c in range(N_CORES):
        b, blk = c // NBLK, c % NBLK
        xb = x[b]
        xT = np.ascontiguousarray(xb.T.astype(BF16))
        xblk = np.ascontiguousarray(xb[blk * BLK : (blk + 1) * BLK])
        xTb = np.ascontiguousarray(xblk.T.astype(BF16))
        msc = ((mask[b].astype(np.float32) != 0).astype(np.float32) * ISCALE).astype(
            BF16
        )
        m = {"xT": xT, "xTb": xTb, "xb": xblk, "msc": msc}
        m.update(w)
        m.update(vecs)
        in_maps.append(m)
    return in_maps


def assemble(per_core_out: list) -> np.ndarray:
    blocks = [np.asarray(o, np.float32) for o in per_core_out]
    full = np.concatenate(blocks, axis=0).reshape(B, S, DIM)
    return full


def kernel(**inputs) -> np.ndarray:
    from concourse.bass_utils import run_bass_kernel_spmd

    nc = _get_nc()
    in_maps = make_in_maps(inputs)
    res = run_bass_kernel_spmd(nc, in_maps, list(range(N_CORES)))
    return assemble([r["out"] for r in res.results])


# revision 20
# speedup vs baseline: 1.0227x; 1.0227x over previous
"""Trainium2 Bass kernel for a dense transformer encoder layer.

Model (faithful to the oracle):
  q,k,v = x@wq+bq, x@wk+bk, x@wv+bv          (12 heads, dk=64, DIM=768)
  scores = q@k^T / sqrt(768)  (note: sqrt(dim_model), not sqrt(dk))
  scores[mask==0] = 1e-11  (NOT -inf; masked keys still contribute ~1/Z)
  attn = softmax(scores); z = attn@v; o = z@wo+bo
  l1 = x + LN(o);  ffn = relu(l1@w1+b1)@w2+b2;  out = l1 + LN(ffn)

Sharding: 4096 tokens (B=2,S=2048) split 8 ways -> 512 tokens/core.
Cores 0-3 own batch 0, cores 4-7 batch 1. K/V are computed for the
core's whole batch (redundantly within each 4-core group): measured
faster than all-gathering them (no collective barrier/trigger stalls,
and the extra matmuls keep the PE array HAM-warm).

Masking trick: mask*1/sqrt(768) is folded into K^T at the K-projection
bias-add (scalar_tensor_tensor: (k+bk)*msc), so masked key columns are
exactly 0 -> score 0 -> exp(0)=1.0 == fp32(exp(1e-11)). This makes the
exp scale-free, so it is batched 3 score-tiles per ACTIVATE (amortizes
the ~350-cycle ScalarE instruction overhead).

Softmax denominator comes from a ones column appended to V (attn@v
with M=65); normalization happens after attn@v via a rank-1 matmul
broadcast of 1/sum.
"""

import math
import os
import sys

import numpy as np

for _p in ("/opt/trn_rl_repo", os.path.expanduser("~/.axon_site/_ro/trn_rl_repo")):
    if os.path.isdir(_p) and _p not in sys.path:
        sys.path.insert(0, _p)

import ml_dtypes  # noqa: E402

BF16 = ml_dtypes.bfloat16

DIM = 768
HEADS = 12
DK = 64
HID = 4 * DIM  # 3072
B, S = 2, 2048
N_CORES = 8
BLK = 512            # tokens per core
NBLK = S // BLK      # 4 blocks per batch
EPS = 1e-5
ISCALE = 1.0 / math.sqrt(DIM)

FT = DIM // 128   # 6 feature tiles
TT = BLK // 128   # 4 token tiles per core block
ST = S // 128     # 16 key tiles per batch
HT = HID // 128   # 24 hidden tiles

# exp batching: groups of score k-tiles fused into one ACTIVATE
EXP_GROUPS = [(0, 3), (3, 6), (6, 9), (9, 12), (12, 15), (15, 16)]
EG = 3  # max group width (PSUM: 3 banks * 2 bufs + z + rb = 8 banks)

_CACHE: dict = {}
MAX_PHASE = int(os.environ.get("BASS_KERNEL_PHASES", "5"))


def _build_program():
    import concourse.bass as bass
    import concourse.mybir as mybir
    import concourse.tile as tile
    from concourse import bacc
    from concourse.masks import make_identity

    f32 = mybir.dt.float32
    bf16 = mybir.dt.bfloat16
    AF = mybir.ActivationFunctionType
    ALU = mybir.AluOpType
    AX = mybir.AxisListType

    nc = bacc.Bacc()

    # ---- per-core DRAM I/O ----
    d_xT = nc.dram_tensor("xT", [DIM, S], bf16, kind="ExternalInput")
    d_xTb = nc.dram_tensor("xTb", [DIM, BLK], bf16, kind="ExternalInput")
    d_xb = nc.dram_tensor("xb", [BLK, DIM], f32, kind="ExternalInput")
    d_msc = nc.dram_tensor("msc", [S], bf16, kind="ExternalInput")
    d_wq = nc.dram_tensor("wq", [DIM, DIM], bf16, kind="ExternalInput")
    d_wk = nc.dram_tensor("wk", [DIM, DIM], bf16, kind="ExternalInput")
    d_wv = nc.dram_tensor("wv", [DIM, DIM], bf16, kind="ExternalInput")
    d_wo = nc.dram_tensor("wo", [DIM, DIM], bf16, kind="ExternalInput")
    d_w1 = nc.dram_tensor("w1", [DIM, HID], bf16, kind="ExternalInput")
    d_w2 = nc.dram_tensor("w2", [HID, DIM], bf16, kind="ExternalInput")
    d_bq = nc.dram_tensor("bq", [DIM], f32, kind="ExternalInput")
    d_bk = nc.dram_tensor("bk", [DIM], f32, kind="ExternalInput")
    d_bv = nc.dram_tensor("bv", [DIM], f32, kind="ExternalInput")
    d_bo = nc.dram_tensor("bo", [DIM], f32, kind="ExternalInput")
    d_b1 = nc.dram_tensor("b1", [HID], f32, kind="ExternalInput")
    d_b2 = nc.dram_tensor("b2", [DIM], f32, kind="ExternalInput")
    d_g1 = nc.dram_tensor("g1", [DIM], f32, kind="ExternalInput")
    d_bb1 = nc.dram_tensor("bb1", [DIM], f32, kind="ExternalInput")
    d_g2 = nc.dram_tensor("g2", [DIM], f32, kind="ExternalInput")
    d_bb2 = nc.dram_tensor("bb2", [DIM], f32, kind="ExternalInput")
    d_out = nc.dram_tensor("out", [BLK, DIM], f32, kind="ExternalOutput")

    def bcast_ap(handle, n=128):
        ap = handle[:]
        return bass.AP(tensor=ap.tensor, offset=ap.offset, ap=[[0, n]] + list(ap.ap))

    with tile.TileContext(nc) as tc:
        with (
            tc.tile_pool(name="const", bufs=1) as const,
            tc.tile_pool(name="bigres", bufs=1) as big,
        ):
            # ---------- constants ----------
            # (bq/bk/b1 DMAs are issued inside phase 1, after the weight
            # loads they follow on the sync queue)
            sb_bk = const.tile([128, FT], f32)
            sb_bq = const.tile([128, FT], f32)
            sb_b1 = const.tile([128, HT], f32)
            bo_bc = const.tile([128, DIM], f32)
            nc.gpsimd.dma_start(out=bo_bc, in_=bcast_ap(d_bo))
            b2_bc = const.tile([128, DIM], f32)
            nc.gpsimd.dma_start(out=b2_bc, in_=bcast_ap(d_b2))
            g1_bc = const.tile([128, DIM], f32)
            nc.gpsimd.dma_start(out=g1_bc, in_=bcast_ap(d_g1))
            bb1_bc = const.tile([128, DIM], f32)
            nc.gpsimd.dma_start(out=bb1_bc, in_=bcast_ap(d_bb1))
            g2_bc = const.tile([128, DIM], f32)
            nc.gpsimd.dma_start(out=g2_bc, in_=bcast_ap(d_g2))
            bb2_bc = const.tile([128, DIM], f32)
            nc.gpsimd.dma_start(out=bb2_bc, in_=bcast_ap(d_bb2))
            ident = const.tile([128, 128], f32)
            make_identity(nc, ident[:])
            ones64 = const.tile([1, 64], f32)
            nc.vector.memset(ones64, 1.0)
            eps_t = const.tile([128, 1], f32)
            nc.vector.memset(eps_t, EPS)

            # ---------- persistent activations ----------
            sb_xblk = big.tile([128, TT, DIM], f32)  # residual x (needed ph3)
            sb_l1 = big.tile([128, TT, DIM], f32)
            sb_zT = big.tile([128, FT, BLK], bf16)  # z^T normalized (ph2 -> ph3)

            # attention-scoped residents (freed before phase 3's LN pools)
            attn_res_cm = tc.tile_pool(name="attn_res", bufs=1)
            attn_res = attn_res_cm.__enter__()
            sb_K = attn_res.tile([128, FT, S], bf16)   # K^T*msc, feat-major
            sb_Q = attn_res.tile([128, FT, BLK], bf16)  # Q^T, feat-major
            sb_V = attn_res.tile([128, ST, HEADS, DK + 1], bf16)  # V + ones col

            # ============ Phase 1: QKV projections ============
            with (
                tc.tile_pool(name="xw", bufs=1) as xw,
                tc.tile_pool(name="ps1", bufs=4, space="PSUM") as ps1,
                tc.tile_pool(name="ps1v", bufs=4, space="PSUM") as ps1v,
            ):
                # phase-1-only broadcast constants
                msc_bc = xw.tile([128, S], bf16)
                nc.gpsimd.dma_start(out=msc_bc, in_=bcast_ap(d_msc))
                bv_bc = xw.tile([128, DIM], f32)
                nc.gpsimd.dma_start(out=bv_bc, in_=bcast_ap(d_bv))
                # critical-path order: Q's inputs (smallest) first, then K's
                # with xT split in halves so K matmuls start early
                w_q = xw.tile([128, FT, DIM], bf16)
                nc.sync.dma_start(
                    out=w_q, in_=d_wq[:].rearrange("(t p) o -> p t o", p=128)
                )
                sb_xTb = xw.tile([128, FT, BLK], bf16)
                nc.sync.dma_start(
                    out=sb_xTb, in_=d_xTb[:].rearrange("(t p) n -> p t n", p=128)
                )
                nc.sync.dma_start(
                    out=sb_bq, in_=d_bq[:].rearrange("(t p) -> p t", p=128)
                )
                w_k = xw.tile([128, FT, DIM], bf16)
                nc.sync.dma_start(
                    out=w_k, in_=d_wk[:].rearrange("(t p) o -> p t o", p=128)
                )
                sb_xT = xw.tile([128, FT, S], bf16)
                nc.sync.dma_start(
                    out=sb_xT[:, :, 0 : S // 2],
                    in_=d_xT[:, 0 : S // 2].rearrange("(t p) n -> p t n", p=128),
                )
                nc.sync.dma_start(
                    out=sb_bk, in_=d_bk[:].rearrange("(t p) -> p t", p=128)
                )
                nc.sync.dma_start(
                    out=sb_xT[:, :, S // 2 : S],
                    in_=d_xT[:, S // 2 : S].rearrange("(t p) n -> p t n", p=128),
                )
                w_v = xw.tile([128, FT, DIM], bf16)
                nc.sync.dma_start(
                    out=w_v, in_=d_wv[:].rearrange("(t p) o -> p t o", p=128)
                )
                # not needed until later phases: queue behind everything
                nc.sync.dma_start(
                    out=sb_xblk, in_=d_xb[:].rearrange("(t p) d -> p t d", p=128)
                )
                nc.sync.dma_start(
                    out=sb_b1, in_=d_b1[:].rearrange("(t p) -> p t", p=128)
                )

                # Q^T feat-major for the core's block (first DMAs to land)
                for ft in range(FT):
                    ps = ps1.tile([128, 512], f32, tag="p")
                    for kt in range(FT):
                        nc.tensor.matmul(
                            ps,
                            w_q[:, kt, ft * 128 : (ft + 1) * 128],
                            sb_xTb[:, kt, :],
                            start=(kt == 0),
                            stop=(kt == FT - 1),
                        )
                    nc.vector.tensor_scalar_add(
                        sb_Q[:, ft, :], ps, sb_bq[:, ft : ft + 1]
                    )
                # K^T feat-major over the whole batch, (K+bk)*msc fused;
                # nt-outer so the first xT half is enough to start
                for nt in range(S // 512):
                    for ft in range(FT):
                        ps = ps1.tile([128, 512], f32, tag="p")
                        for kt in range(FT):
                            nc.tensor.matmul(
                                ps,
                                w_k[:, kt, ft * 128 : (ft + 1) * 128],
                                sb_xT[:, kt, nt * 512 : (nt + 1) * 512],
                                start=(kt == 0),
                                stop=(kt == FT - 1),
                            )
                        nc.vector.scalar_tensor_tensor(
                            out=sb_K[:, ft, nt * 512 : (nt + 1) * 512],
                            in0=ps,
                            scalar=sb_bk[:, ft : ft + 1],
                            in1=msc_bc[:, nt * 512 : (nt + 1) * 512],
                            op0=ALU.add,
                            op1=ALU.mult,
                        )
                # V tok-major over the whole batch, laid out [tok, head, dk+1]
                nc.vector.memset(sb_V[:, :, :, DK : DK + 1], 1.0)
                for nh in range(2):
                    for tt in range(ST):
                        ps = ps1v.tile([128, 384], f32, tag="vp")
                        for kt in range(FT):
                            nc.tensor.matmul(
                                ps,
                                sb_xT[:, kt, tt * 128 : (tt + 1) * 128],
                                w_v[:, kt, nh * 384 : (nh + 1) * 384],
                                start=(kt == 0),
                                stop=(kt == FT - 1),
                            )
                        nc.vector.scalar_tensor_tensor(
                            out=sb_V[:, tt, nh * 6 : (nh + 1) * 6, 0:DK],
                            in0=ps[:].rearrange("p (h d) -> p h d", d=DK),
                            scalar=1.0,
                            in1=bv_bc[:, nh * 384 : (nh + 1) * 384].rearrange(
                                "p (h d) -> p h d", d=DK
                            ),
                            op0=ALU.mult,
                            op1=ALU.add,
                        )

            # prefetch wo now -- the DMA overlaps the attention phase
            w_o = big.tile([128, FT, DIM], bf16)
            nc.sync.dma_start(out=w_o, in_=d_wo[:].rearrange("(t p) o -> p t o", p=128))

            if MAX_PHASE >= 2:
                # ============ Phase 2: attention ============
                # PSUM budget: ps_sc 3 banks x 2 bufs + ps_z 1 x 2 bufs = 8.
                # The rank-1 1/Z broadcast borrows a ps_sc buffer briefly.
                with (
                    tc.tile_pool(name="expp", bufs=16) as expp,
                    tc.tile_pool(name="attsm", bufs=3) as attsm,
                    tc.tile_pool(name="ps_sc", bufs=2, space="PSUM") as ps_sc,
                    tc.tile_pool(name="ps_z", bufs=2, space="PSUM") as ps_z,
                ):
                    def z_chunk(zps, ets, hp, half, a, b):
                        h = 2 * hp + half
                        for kt2 in range(a, b):
                            nc.tensor.matmul(
                                zps[half][0 : DK + 1, :],
                                sb_V[:, kt2, h, :],
                                ets[half][kt2],
                                start=(kt2 == 0),
                                stop=(kt2 == ST - 1),
                            )

                    def z_tail(zps, ht, half):
                        # 1/Z broadcast lives in partitions 64-127 of the z
                        # tile's own PSUM bank (row 64, the denominator, is
                        # consumed by the reciprocal before being overwritten)
                        ho = half * 64
                        zp = zps[half]
                        rsum = attsm.tile([1, BLK], f32, tag="rsum")
                        nc.vector.reciprocal(rsum, zp[DK : DK + 1, :])
                        nc.tensor.matmul(
                            zp[64:128, :], ones64[:], rsum, start=True, stop=True
                        )
                        rb = attsm.tile([64, BLK], f32, tag="rbs")
                        nc.vector.tensor_copy(rb, zp[64:128, :])
                        nc.vector.tensor_mul(
                            sb_zT[ho : ho + 64, ht, :], zp[0:DK, :], rb
                        )

                    for hp in range(HEADS // 2):
                        ht = hp
                        # The two heads of a pair use disjoint PE row groups
                        # (partitions 0-63 / 64-127): adjacent-emitted score
                        # matmuls for the two halves run concurrently in the
                        # array. z-matmul chunks are staggered one exp-group
                        # behind the scores so the PE always has runnable
                        # work while ScalarE exps.
                        ets = ([], [])
                        zps = [
                            ps_z.tile([128, BLK], f32, tag="z", name="zp0"),
                            ps_z.tile([128, BLK], f32, tag="z", name="zp1"),
                        ]
                        for gi, (a, b) in enumerate(EXP_GROUPS):
                            g = b - a
                            pss = [
                                ps_sc.tile([128, EG * 512], f32, tag="sc", name="ps0"),
                                ps_sc.tile([128, EG * 512], f32, tag="sc", name="ps1"),
                            ]
                            for j, kt2 in enumerate(range(a, b)):
                                for half in (0, 1):
                                    ho = half * 64
                                    nc.tensor.matmul(
                                        pss[half][:, j * 512 : (j + 1) * 512],
                                        sb_K[ho : ho + 64, ht, kt2 * 128 : (kt2 + 1) * 128],
                                        sb_Q[ho : ho + 64, ht, :],
                                        start=True,
                                        stop=True,
                                    )
                            for half in (0, 1):
                                et = expp.tile([128, EG * 512], bf16, tag="exp")
                                nc.scalar.activation(
                                    et[:, : g * 512], pss[half][:, : g * 512], AF.Exp
                                )
                                for j in range(g):
                                    ets[half].append(et[:, j * 512 : (j + 1) * 512])
                            if gi >= 1:
                                pa, pb = EXP_GROUPS[gi - 1]
                                for half in (0, 1):
                                    z_chunk(zps, ets, hp, half, pa, pb)
                        la, lb = EXP_GROUPS[-1]
                        for half in (0, 1):
                            z_chunk(zps, ets, hp, half, la, lb)
                            z_tail(zps, ht, half)

            attn_res_cm.__exit__(None, None, None)

            if MAX_PHASE >= 3:
                # ============ Phase 3: O proj + LN1 (+residual) ============
                def layer_norm_to(out_ap, x_ap, g_bc_t, resid_ap, pool):
                    s = pool.tile([128, 1], f32, tag="ln_s")
                    nc.vector.tensor_reduce(s, x_ap, axis=AX.X, op=ALU.add)
                    mean = pool.tile([128, 1], f32, tag="ln_m")
                    nc.scalar.mul(mean, s, 1.0 / DIM)
                    xc = pool.tile([128, DIM], f32, tag="ln_xc")
                    nc.vector.tensor_scalar(xc, x_ap, mean, None, op0=ALU.subtract)
                    junk = pool.tile([128, DIM], f32, tag="ln_j")
                    var = pool.tile([128, 1], f32, tag="ln_v")
                    # (tensor_tensor_reduce crashes the device on this runtime;
                    # scalar_tensor_tensor with accum_out works)
                    nc.vector.scalar_tensor_tensor(
                        out=junk, in0=xc, scalar=1.0, in1=xc,
                        op0=ALU.mult, op1=ALU.mult, accum_out=var,
                    )
                    nc.vector.tensor_scalar_mul(var, var, 1.0 / DIM)
                    sd = pool.tile([128, 1], f32, tag="ln_sd")
                    nc.scalar.activation(sd, var, AF.Sqrt, bias=eps_t[:])
                    rstd = pool.tile([128, 1], f32, tag="ln_r")
                    nc.vector.reciprocal(rstd, sd)
                    t = pool.tile([128, DIM], f32, tag="ln_t")
                    nc.vector.tensor_scalar(t, xc, rstd, None, op0=ALU.mult)
                    tg = pool.tile([128, DIM], f32, tag="ln_tg")
                    nc.vector.tensor_mul(tg, t, g_bc_t)
                    nc.vector.tensor_add(out_ap, tg, resid_ap)

                ffnp_cm = tc.tile_pool(name="ffnp", bufs=1)
                ffnp = ffnp_cm.__enter__()
                # w1 first (feeds FFN1 soon), w2 lands during FFN1; both
                # DMAs overlap phase 3's O-projection + LayerNorm
                w1_t = ffnp.tile([128, FT, HID], bf16)
                nc.sync.dma_start(
                    out=w1_t, in_=d_w1[:].rearrange("(t p) h -> p t h", p=128)
                )
                w2_t = ffnp.tile([128, HT, DIM], bf16)
                nc.sync.dma_start(
                    out=w2_t, in_=d_w2[:].rearrange("(t p) o -> p t o", p=128)
                )
                sb_hT = ffnp.tile([128, HT, BLK], bf16)  # relu(ffn1)^T
                sb_l1T = ffnp.tile([128, FT, BLK], bf16)

                with (
                    tc.tile_pool(name="ln1p", bufs=2) as ln1p,
                    tc.tile_pool(name="ps_o", bufs=4, space="PSUM") as ps_o,
                    tc.tile_pool(name="ps_t1", bufs=2, space="PSUM") as ps_t1,
                ):
                    for tt in range(TT):
                        l1pre = ln1p.tile([128, DIM], f32, tag="l1pre")
                        for nh in range(2):
                            ps = ps_o.tile([128, 384], f32, tag="op")
                            for kt in range(FT):
                                nc.tensor.matmul(
                                    ps,
                                    sb_zT[:, kt, tt * 128 : (tt + 1) * 128],
                                    w_o[:, kt, nh * 384 : (nh + 1) * 384],
                                    start=(kt == 0),
                                    stop=(kt == FT - 1),
                                )
                            nc.vector.scalar_tensor_tensor(
                                out=l1pre[:, nh * 384 : (nh + 1) * 384],
                                in0=ps,
                                scalar=1.0,
                                in1=bo_bc[:, nh * 384 : (nh + 1) * 384],
                                op0=ALU.mult,
                                op1=ALU.add,
                            )
                        xb1 = ln1p.tile([128, DIM], f32, tag="xb1")
                        nc.vector.tensor_add(xb1, sb_xblk[:, tt, :], bb1_bc)
                        layer_norm_to(sb_l1[:, tt, :], l1pre[:], g1_bc, xb1, ln1p)
                        # transpose this l1 tile immediately (feeds FFN1)
                        for ft in range(FT):
                            pst = ps_t1.tile([128, 128], f32, tag="tp")
                            nc.tensor.transpose(
                                pst, sb_l1[:, tt, ft * 128 : (ft + 1) * 128], ident[:]
                            )
                            nc.scalar.copy(
                                sb_l1T[:, ft, tt * 128 : (tt + 1) * 128], pst
                            )

            if MAX_PHASE >= 4:
                # ============ Phase 4+5: FFN (weights already resident) ====
                with (
                    tc.tile_pool(name="ln2p", bufs=2) as ln2p,
                    tc.tile_pool(name="outp", bufs=3) as outp,
                    tc.tile_pool(name="ps_f1", bufs=4, space="PSUM") as ps_f1,
                ):
                    for ht2 in range(HT):
                        ps = ps_f1.tile([128, BLK], f32, tag="f1")
                        for kt in range(FT):
                            nc.tensor.matmul(
                                ps,
                                w1_t[:, kt, ht2 * 128 : (ht2 + 1) * 128],
                                sb_l1T[:, kt, :],
                                start=(kt == 0),
                                stop=(kt == FT - 1),
                            )
                        # relu(x + b1) on DVE: (x add b1) max 0
                        nc.vector.tensor_scalar(
                            sb_hT[:, ht2, :], ps, sb_b1[:, ht2 : ht2 + 1], 0.0,
                            op0=ALU.add, op1=ALU.max,
                        )

                    if MAX_PHASE >= 5:
                        # ============ Phase 5: FFN2 + LN2 + out ============
                        out_r = d_out[:].rearrange("(t p) d -> p t d", p=128)
                        for tt in range(TT):
                            f2pre = ln2p.tile([128, DIM], f32, tag="f2pre")
                            for nh in range(2):
                                ps = ps_f1.tile([128, 384], f32, tag="f2")
                                for kt in range(HT):
                                    nc.tensor.matmul(
                                        ps,
                                        sb_hT[:, kt, tt * 128 : (tt + 1) * 128],
                                        w2_t[:, kt, nh * 384 : (nh + 1) * 384],
                                        start=(kt == 0),
                                        stop=(kt == HT - 1),
                                    )
                                nc.vector.scalar_tensor_tensor(
                                    out=f2pre[:, nh * 384 : (nh + 1) * 384],
                                    in0=ps,
                                    scalar=1.0,
                                    in1=b2_bc[:, nh * 384 : (nh + 1) * 384],
                                    op0=ALU.mult,
                                    op1=ALU.add,
                                )
                            l1b = ln2p.tile([128, DIM], f32, tag="l1b")
                            nc.vector.tensor_add(l1b, sb_l1[:, tt, :], bb2_bc)
                            o_sb = outp.tile([128, DIM], f32, tag="osb")
                            layer_norm_to(o_sb[:], f2pre[:], g2_bc, l1b, ln2p)
                            nc.sync.dma_start(out=out_r[:, tt, :], in_=o_sb)

            if MAX_PHASE >= 3:
                ffnp_cm.__exit__(None, None, None)

    return nc


def _get_nc(finalized=True):
    if "nc" not in _CACHE:
        _CACHE["nc"] = _build_program()
    nc = _CACHE["nc"]
    if finalized and not nc.is_finalized():
        nc.finalize()
    return nc


def make_in_maps(inputs: dict) -> list:
    x = np.asarray(inputs["x_n"], np.float32).reshape(B, S, DIM)
    mask = np.asarray(inputs["mask"]).reshape(B, S)
    w = {
        k: np.ascontiguousarray(np.asarray(inputs[k], np.float32).astype(BF16))
        for k in ("wq", "wk", "wv", "wo", "w1", "w2")
    }
    vecs = {
        "bq": inputs["bq"], "bk": inputs["bk"], "bv": inputs["bv"],
        "bo": inputs["bo"], "b1": inputs["b1"], "b2": inputs["b2"],
        "g1": inputs["ln1_g"], "bb1": inputs["ln1_b"],
        "g2": inputs["ln2_g"], "bb2": inputs["ln2_b"],
    }
    vecs = {k: np.ascontiguousarray(np.asarray(v, np.float32)) for k, v in vecs.items()}
    in_maps = []
    for c in range(N_CORES):
        b, blk = c // NBLK, c % NBLK
        xb = x[b]
        xT = np.ascontiguousarray(xb.T.astype(BF16))
        xblk = np.ascontiguousarray(xb[blk * BLK : (blk + 1) * BLK])
        xTb = np.ascontiguousarray(xblk.T.astype(BF16))
        msc = ((mask[b].astype(np.float32) != 0).astype(np.float32) * ISCALE).astype(
            BF16
        )
        m = {"xT": xT, "xTb": xTb, "xb": xblk, "msc": msc}
        m.update(w)
        m.update(vecs)
        in_maps.append(m)
    return in_maps


def assemble(per_core_out: list) -> np.ndarray:
    blocks = [np.asarray(o, np.float32) for o in per_core_out]
    full = np.concatenate(blocks, axis=0).reshape(B, S, DIM)
    return full


def kernel(**inputs) -> np.ndarray:
    from concourse.bass_utils import run_bass_kernel_spmd

    nc = _get_nc()
    in_maps = make_in_maps(inputs)
    res = run_bass_kernel_spmd(nc, in_maps, list(range(N_CORES)))
    return assemble([r["out"] for r in res.results])


# revision 21
# speedup vs baseline: 1.0685x; 1.0448x over previous
"""Trainium2 Bass kernel for a dense transformer encoder layer.

Model (faithful to the oracle):
  q,k,v = x@wq+bq, x@wk+bk, x@wv+bv          (12 heads, dk=64, DIM=768)
  scores = q@k^T / sqrt(768)  (note: sqrt(dim_model), not sqrt(dk))
  scores[mask==0] = 1e-11  (NOT -inf; masked keys still contribute ~1/Z)
  attn = softmax(scores); z = attn@v; o = z@wo+bo
  l1 = x + LN(o);  ffn = relu(l1@w1+b1)@w2+b2;  out = l1 + LN(ffn)

Sharding: 4096 tokens (B=2,S=2048) split 8 ways -> 512 tokens/core.
Cores 0-3 own batch 0, cores 4-7 batch 1. K/V are computed for the
core's whole batch (redundantly within each 4-core group): measured
faster than all-gathering them (no collective barrier/trigger stalls,
and the extra matmuls keep the PE array HAM-warm).

Masking trick: mask*1/sqrt(768) is folded into K^T at the K-projection
bias-add (scalar_tensor_tensor: (k+bk)*msc), so masked key columns are
exactly 0 -> score 0 -> exp(0)=1.0 == fp32(exp(1e-11)). This makes the
exp scale-free, so it is batched 3 score-tiles per ACTIVATE (amortizes
the ~350-cycle ScalarE instruction overhead).

Softmax denominator comes from a ones column appended to V (attn@v
with M=65); normalization happens after attn@v via a rank-1 matmul
broadcast of 1/sum.
"""

import math
import os
import sys

import numpy as np

for _p in ("/opt/trn_rl_repo", os.path.expanduser("~/.axon_site/_ro/trn_rl_repo")):
    if os.path.isdir(_p) and _p not in sys.path:
        sys.path.insert(0, _p)

import ml_dtypes  # noqa: E402

BF16 = ml_dtypes.bfloat16

DIM = 768
HEADS = 12
DK = 64
HID = 4 * DIM  # 3072
B, S = 2, 2048
N_CORES = 8
BLK = 512            # tokens per core
NBLK = S // BLK      # 4 blocks per batch
EPS = 1e-5
ISCALE = 1.0 / math.sqrt(DIM)

FT = DIM // 128   # 6 feature tiles
TT = BLK // 128   # 4 token tiles per core block
ST = S // 128     # 16 key tiles per batch
HT = HID // 128   # 24 hidden tiles

# exp batching: groups of score k-tiles fused into one ACTIVATE
EXP_GROUPS = [(0, 3), (3, 6), (6, 9), (9, 12), (12, 15), (15, 16)]
EG = 3  # max group width (PSUM: 3 banks * 2 bufs + z + rb = 8 banks)

_CACHE: dict = {}
MAX_PHASE = int(os.environ.get("BASS_KERNEL_PHASES", "5"))


def _build_program():
    import concourse.bass as bass
    import concourse.mybir as mybir
    import concourse.tile as tile
    from concourse import bacc
    from concourse.masks import make_identity

    f32 = mybir.dt.float32
    bf16 = mybir.dt.bfloat16
    AF = mybir.ActivationFunctionType
    ALU = mybir.AluOpType
    AX = mybir.AxisListType

    nc = bacc.Bacc()

    # ---- per-core DRAM I/O ----
    d_xT = nc.dram_tensor("xT", [DIM, S], bf16, kind="ExternalInput")
    d_xTb = nc.dram_tensor("xTb", [DIM, BLK], bf16, kind="ExternalInput")
    d_xb = nc.dram_tensor("xb", [BLK, DIM], f32, kind="ExternalInput")
    d_msc = nc.dram_tensor("msc", [S], bf16, kind="ExternalInput")
    d_wq = nc.dram_tensor("wq", [DIM, DIM], bf16, kind="ExternalInput")
    d_wk = nc.dram_tensor("wk", [DIM, DIM], bf16, kind="ExternalInput")
    d_wv = nc.dram_tensor("wv", [DIM, DIM], bf16, kind="ExternalInput")
    d_wo = nc.dram_tensor("wo", [DIM, DIM], bf16, kind="ExternalInput")
    d_w1 = nc.dram_tensor("w1", [DIM, HID], bf16, kind="ExternalInput")
    d_w2 = nc.dram_tensor("w2", [HID, DIM], bf16, kind="ExternalInput")
    d_bq = nc.dram_tensor("bq", [DIM], f32, kind="ExternalInput")
    d_bk = nc.dram_tensor("bk", [DIM], f32, kind="ExternalInput")
    d_bv = nc.dram_tensor("bv", [DIM], f32, kind="ExternalInput")
    d_bo = nc.dram_tensor("bo", [DIM], f32, kind="ExternalInput")
    d_b1 = nc.dram_tensor("b1", [HID], f32, kind="ExternalInput")
    d_b2 = nc.dram_tensor("b2", [DIM], f32, kind="ExternalInput")
    d_g1 = nc.dram_tensor("g1", [DIM], f32, kind="ExternalInput")
    d_bb1 = nc.dram_tensor("bb1", [DIM], f32, kind="ExternalInput")
    d_g2 = nc.dram_tensor("g2", [DIM], f32, kind="ExternalInput")
    d_bb2 = nc.dram_tensor("bb2", [DIM], f32, kind="ExternalInput")
    d_out = nc.dram_tensor("out", [BLK, DIM], f32, kind="ExternalOutput")

    def bcast_ap(handle, n=128):
        ap = handle[:]
        return bass.AP(tensor=ap.tensor, offset=ap.offset, ap=[[0, n]] + list(ap.ap))

    with tile.TileContext(nc) as tc:
        with (
            tc.tile_pool(name="const", bufs=1) as const,
            tc.tile_pool(name="bigres", bufs=1) as big,
        ):
            # ---------- constants ----------
            # (bq/bk/b1 DMAs are issued inside phase 1, after the weight
            # loads they follow on the sync queue)
            sb_bk = const.tile([128, FT], f32)
            sb_bq = const.tile([128, FT], f32)
            sb_b1 = const.tile([128, HT], f32)
            bo_bc = const.tile([128, DIM], f32)
            nc.gpsimd.dma_start(out=bo_bc, in_=bcast_ap(d_bo))
            b2_bc = const.tile([128, DIM], f32)
            nc.gpsimd.dma_start(out=b2_bc, in_=bcast_ap(d_b2))
            g1_bc = const.tile([128, DIM], f32)
            nc.gpsimd.dma_start(out=g1_bc, in_=bcast_ap(d_g1))
            bb1_bc = const.tile([128, DIM], f32)
            nc.gpsimd.dma_start(out=bb1_bc, in_=bcast_ap(d_bb1))
            g2_bc = const.tile([128, DIM], f32)
            nc.gpsimd.dma_start(out=g2_bc, in_=bcast_ap(d_g2))
            bb2_bc = const.tile([128, DIM], f32)
            nc.gpsimd.dma_start(out=bb2_bc, in_=bcast_ap(d_bb2))
            ident = const.tile([128, 128], f32)
            make_identity(nc, ident[:])
            ones64 = const.tile([1, 64], f32)
            nc.vector.memset(ones64, 1.0)
            eps_t = const.tile([128, 1], f32)
            nc.vector.memset(eps_t, EPS)

            # ---------- persistent activations ----------
            sb_xblk = big.tile([128, TT, DIM], f32)  # residual x (needed ph3)
            sb_l1 = big.tile([128, TT, DIM], f32)
            sb_zT = big.tile([128, FT, BLK], bf16)  # z^T normalized (ph2 -> ph3)

            # attention-scoped residents (freed before phase 3's LN pools)
            attn_res_cm = tc.tile_pool(name="attn_res", bufs=1)
            attn_res = attn_res_cm.__enter__()
            sb_K = attn_res.tile([128, FT, S], bf16)   # K^T*msc, feat-major
            sb_Q = attn_res.tile([128, FT, BLK], bf16)  # Q^T, feat-major
            sb_V = attn_res.tile([128, ST, HEADS, DK + 1], bf16)  # V + ones col

            # ============ Phase 1: QKV projections ============
            with (
                tc.tile_pool(name="xw", bufs=1) as xw,
                tc.tile_pool(name="ps1", bufs=4, space="PSUM") as ps1,
                tc.tile_pool(name="ps1v", bufs=4, space="PSUM") as ps1v,
            ):
                # phase-1-only broadcast constants
                msc_bc = xw.tile([128, S], bf16)
                nc.gpsimd.dma_start(out=msc_bc, in_=bcast_ap(d_msc))
                bv_bc = xw.tile([128, DIM], f32)
                nc.gpsimd.dma_start(out=bv_bc, in_=bcast_ap(d_bv))
                # critical-path order: Q's inputs (smallest) first, then K's
                # with xT split in halves so K matmuls start early
                w_q = xw.tile([128, FT, DIM], bf16)
                nc.sync.dma_start(
                    out=w_q, in_=d_wq[:].rearrange("(t p) o -> p t o", p=128)
                )
                sb_xTb = xw.tile([128, FT, BLK], bf16)
                nc.sync.dma_start(
                    out=sb_xTb, in_=d_xTb[:].rearrange("(t p) n -> p t n", p=128)
                )
                nc.sync.dma_start(
                    out=sb_bq, in_=d_bq[:].rearrange("(t p) -> p t", p=128)
                )
                w_k = xw.tile([128, FT, DIM], bf16)
                nc.sync.dma_start(
                    out=w_k, in_=d_wk[:].rearrange("(t p) o -> p t o", p=128)
                )
                sb_xT = xw.tile([128, FT, S], bf16)
                nc.sync.dma_start(
                    out=sb_xT[:, :, 0 : S // 2],
                    in_=d_xT[:, 0 : S // 2].rearrange("(t p) n -> p t n", p=128),
                )
                nc.sync.dma_start(
                    out=sb_bk, in_=d_bk[:].rearrange("(t p) -> p t", p=128)
                )
                nc.sync.dma_start(
                    out=sb_xT[:, :, S // 2 : S],
                    in_=d_xT[:, S // 2 : S].rearrange("(t p) n -> p t n", p=128),
                )
                w_v = xw.tile([128, FT, DIM], bf16)
                nc.sync.dma_start(
                    out=w_v, in_=d_wv[:].rearrange("(t p) o -> p t o", p=128)
                )
                # not needed until later phases: queue behind everything
                nc.sync.dma_start(
                    out=sb_xblk, in_=d_xb[:].rearrange("(t p) d -> p t d", p=128)
                )
                nc.sync.dma_start(
                    out=sb_b1, in_=d_b1[:].rearrange("(t p) -> p t", p=128)
                )

                # Q^T feat-major for the core's block (first DMAs to land)
                for ft in range(FT):
                    ps = ps1.tile([128, 512], f32, tag="p")
                    for kt in range(FT):
                        nc.tensor.matmul(
                            ps,
                            w_q[:, kt, ft * 128 : (ft + 1) * 128],
                            sb_xTb[:, kt, :],
                            start=(kt == 0),
                            stop=(kt == FT - 1),
                        )
                    nc.vector.tensor_scalar_add(
                        sb_Q[:, ft, :], ps, sb_bq[:, ft : ft + 1]
                    )
                # K^T feat-major over the whole batch, (K+bk)*msc fused;
                # nt-outer so the first xT half is enough to start
                for nt in range(S // 512):
                    for ft in range(FT):
                        ps = ps1.tile([128, 512], f32, tag="p")
                        for kt in range(FT):
                            nc.tensor.matmul(
                                ps,
                                w_k[:, kt, ft * 128 : (ft + 1) * 128],
                                sb_xT[:, kt, nt * 512 : (nt + 1) * 512],
                                start=(kt == 0),
                                stop=(kt == FT - 1),
                            )
                        nc.vector.scalar_tensor_tensor(
                            out=sb_K[:, ft, nt * 512 : (nt + 1) * 512],
                            in0=ps,
                            scalar=sb_bk[:, ft : ft + 1],
                            in1=msc_bc[:, nt * 512 : (nt + 1) * 512],
                            op0=ALU.add,
                            op1=ALU.mult,
                        )
                # V tok-major over the whole batch, laid out [tok, head, dk+1]
                nc.vector.memset(sb_V[:, :, :, DK : DK + 1], 1.0)
                for nh in range(2):
                    for tt in range(ST):
                        ps = ps1v.tile([128, 384], f32, tag="vp")
                        for kt in range(FT):
                            nc.tensor.matmul(
                                ps,
                                sb_xT[:, kt, tt * 128 : (tt + 1) * 128],
                                w_v[:, kt, nh * 384 : (nh + 1) * 384],
                                start=(kt == 0),
                                stop=(kt == FT - 1),
                            )
                        nc.vector.scalar_tensor_tensor(
                            out=sb_V[:, tt, nh * 6 : (nh + 1) * 6, 0:DK],
                            in0=ps[:].rearrange("p (h d) -> p h d", d=DK),
                            scalar=1.0,
                            in1=bv_bc[:, nh * 384 : (nh + 1) * 384].rearrange(
                                "p (h d) -> p h d", d=DK
                            ),
                            op0=ALU.mult,
                            op1=ALU.add,
                        )

            # prefetch wo now -- the DMA overlaps the attention phase
            w_o = big.tile([128, FT, DIM], bf16)
            nc.sync.dma_start(out=w_o, in_=d_wo[:].rearrange("(t p) o -> p t o", p=128))

            if MAX_PHASE >= 2:
                # ============ Phase 2: attention ============
                # PSUM budget: ps_sc 3 banks x 2 bufs + ps_z 1 x 2 bufs = 8.
                # The rank-1 1/Z broadcast borrows a ps_sc buffer briefly.
                with (
                    tc.tile_pool(name="expp", bufs=16) as expp,
                    tc.tile_pool(name="attsm", bufs=3) as attsm,
                    tc.tile_pool(name="ps_sc", bufs=2, space="PSUM") as ps_sc,
                    tc.tile_pool(name="ps_z", bufs=2, space="PSUM") as ps_z,
                ):
                    def z_chunk(zps, ets, hp, half, a, b):
                        h = 2 * hp + half
                        for kt2 in range(a, b):
                            nc.tensor.matmul(
                                zps[half][0 : DK + 1, :],
                                sb_V[:, kt2, h, :],
                                ets[half][kt2],
                                start=(kt2 == 0),
                                stop=(kt2 == ST - 1),
                            )

                    def z_tail(zps, ht, half):
                        # 1/Z broadcast lives in partitions 64-127 of the z
                        # tile's own PSUM bank (row 64, the denominator, is
                        # consumed by the reciprocal before being overwritten)
                        ho = half * 64
                        zp = zps[half]
                        rsum = attsm.tile([1, BLK], f32, tag="rsum")
                        nc.vector.reciprocal(rsum, zp[DK : DK + 1, :])
                        nc.tensor.matmul(
                            zp[64:128, :], ones64[:], rsum, start=True, stop=True
                        )
                        rb = attsm.tile([64, BLK], f32, tag="rbs")
                        nc.vector.tensor_copy(rb, zp[64:128, :])
                        nc.vector.tensor_mul(
                            sb_zT[ho : ho + 64, ht, :], zp[0:DK, :], rb
                        )

                    # Flat software pipeline over (pair, exp-group) rounds:
                    # z-matmul chunks run TWO rounds behind their scores so
                    # every PE instruction's inputs (exp tiles) are ready
                    # long before it issues -- a gap-free PE stream lets the
                    # HAM clock-gate reach (and keep) the 2.4 GHz state.
                    NP = HEADS // 2
                    seq = [(hp, gi) for hp in range(NP) for gi in range(len(EXP_GROUPS))]
                    all_ets = [([], []) for _ in range(NP)]
                    all_zps = [None] * NP
                    LAG = 2
                    for idx in range(len(seq) + LAG):
                        if idx < len(seq):
                            hp, gi = seq[idx]
                            ht = hp
                            a, b = EXP_GROUPS[gi]
                            g = b - a
                            if gi == 0:
                                all_zps[hp] = [
                                    ps_z.tile([128, BLK], f32, tag="z", name="zp0"),
                                    ps_z.tile([128, BLK], f32, tag="z", name="zp1"),
                                ]
                            ets = all_ets[hp]
                            for half in (0, 1):
                                ho = half * 64
                                ps = ps_sc.tile(
                                    [128, EG * 512], f32, tag="sc", name="psg"
                                )
                                for j, kt2 in enumerate(range(a, b)):
                                    nc.tensor.matmul(
                                        ps[:, j * 512 : (j + 1) * 512],
                                        sb_K[ho : ho + 64, ht, kt2 * 128 : (kt2 + 1) * 128],
                                        sb_Q[ho : ho + 64, ht, :],
                                        start=True,
                                        stop=True,
                                    )
                                et = expp.tile([128, EG * 512], bf16, tag="exp")
                                nc.scalar.activation(
                                    et[:, : g * 512], ps[:, : g * 512], AF.Exp
                                )
                                for j in range(g):
                                    ets[half].append(et[:, j * 512 : (j + 1) * 512])
                        if idx >= LAG:
                            hp2, gi2 = seq[idx - LAG]
                            pa, pb = EXP_GROUPS[gi2]
                            for half in (0, 1):
                                z_chunk(all_zps[hp2], all_ets[hp2], hp2, half, pa, pb)
                            if gi2 == len(EXP_GROUPS) - 1:
                                for half in (0, 1):
                                    z_tail(all_zps[hp2], hp2, half)

            attn_res_cm.__exit__(None, None, None)

            if MAX_PHASE >= 3:
                # ============ Phase 3: O proj + LN1 (+residual) ============
                def layer_norm_to(out_ap, x_ap, g_bc_t, resid_ap, pool):
                    s = pool.tile([128, 1], f32, tag="ln_s")
                    nc.vector.tensor_reduce(s, x_ap, axis=AX.X, op=ALU.add)
                    mean = pool.tile([128, 1], f32, tag="ln_m")
                    nc.scalar.mul(mean, s, 1.0 / DIM)
                    xc = pool.tile([128, DIM], f32, tag="ln_xc")
                    nc.vector.tensor_scalar(xc, x_ap, mean, None, op0=ALU.subtract)
                    junk = pool.tile([128, DIM], f32, tag="ln_j")
                    var = pool.tile([128, 1], f32, tag="ln_v")
                    # (tensor_tensor_reduce crashes the device on this runtime;
                    # scalar_tensor_tensor with accum_out works)
                    nc.vector.scalar_tensor_tensor(
                        out=junk, in0=xc, scalar=1.0, in1=xc,
                        op0=ALU.mult, op1=ALU.mult, accum_out=var,
                    )
                    nc.vector.tensor_scalar_mul(var, var, 1.0 / DIM)
                    sd = pool.tile([128, 1], f32, tag="ln_sd")
                    nc.scalar.activation(sd, var, AF.Sqrt, bias=eps_t[:])
                    rstd = pool.tile([128, 1], f32, tag="ln_r")
                    nc.vector.reciprocal(rstd, sd)
                    t = pool.tile([128, DIM], f32, tag="ln_t")
                    nc.vector.tensor_scalar(t, xc, rstd, None, op0=ALU.mult)
                    tg = pool.tile([128, DIM], f32, tag="ln_tg")
                    nc.vector.tensor_mul(tg, t, g_bc_t)
                    nc.vector.tensor_add(out_ap, tg, resid_ap)

                ffnp_cm = tc.tile_pool(name="ffnp", bufs=1)
                ffnp = ffnp_cm.__enter__()
                # w1 first (feeds FFN1 soon), w2 lands during FFN1; both
                # DMAs overlap phase 3's O-projection + LayerNorm
                w1_t = ffnp.tile([128, FT, HID], bf16)
                nc.sync.dma_start(
                    out=w1_t, in_=d_w1[:].rearrange("(t p) h -> p t h", p=128)
                )
                w2_t = ffnp.tile([128, HT, DIM], bf16)
                nc.sync.dma_start(
                    out=w2_t, in_=d_w2[:].rearrange("(t p) o -> p t o", p=128)
                )
                sb_hT = ffnp.tile([128, HT, BLK], bf16)  # relu(ffn1)^T
                sb_l1T = ffnp.tile([128, FT, BLK], bf16)

                with (
                    tc.tile_pool(name="ln1p", bufs=2) as ln1p,
                    tc.tile_pool(name="ps_o", bufs=4, space="PSUM") as ps_o,
                    tc.tile_pool(name="ps_t1", bufs=2, space="PSUM") as ps_t1,
                ):
                    for tt in range(TT):
                        l1pre = ln1p.tile([128, DIM], f32, tag="l1pre")
                        for nh in range(2):
                            ps = ps_o.tile([128, 384], f32, tag="op")
                            for kt in range(FT):
                                nc.tensor.matmul(
                                    ps,
                                    sb_zT[:, kt, tt * 128 : (tt + 1) * 128],
                                    w_o[:, kt, nh * 384 : (nh + 1) * 384],
                                    start=(kt == 0),
                                    stop=(kt == FT - 1),
                                )
                            nc.vector.scalar_tensor_tensor(
                                out=l1pre[:, nh * 384 : (nh + 1) * 384],
                                in0=ps,
                                scalar=1.0,
                                in1=bo_bc[:, nh * 384 : (nh + 1) * 384],
                                op0=ALU.mult,
                                op1=ALU.add,
                            )
                        xb1 = ln1p.tile([128, DIM], f32, tag="xb1")
                        nc.vector.tensor_add(xb1, sb_xblk[:, tt, :], bb1_bc)
                        layer_norm_to(sb_l1[:, tt, :], l1pre[:], g1_bc, xb1, ln1p)
                        # transpose this l1 tile immediately (feeds FFN1)
                        for ft in range(FT):
                            pst = ps_t1.tile([128, 128], f32, tag="tp")
                            nc.tensor.transpose(
                                pst, sb_l1[:, tt, ft * 128 : (ft + 1) * 128], ident[:]
                            )
                            nc.scalar.copy(
                                sb_l1T[:, ft, tt * 128 : (tt + 1) * 128], pst
                            )

            if MAX_PHASE >= 4:
                # ============ Phase 4+5: FFN (weights already resident) ====
                with (
                    tc.tile_pool(name="ln2p", bufs=2) as ln2p,
                    tc.tile_pool(name="outp", bufs=3) as outp,
                    tc.tile_pool(name="ps_f1", bufs=4, space="PSUM") as ps_f1,
                ):
                    for ht2 in range(HT):
                        ps = ps_f1.tile([128, BLK], f32, tag="f1")
                        for kt in range(FT):
                            nc.tensor.matmul(
                                ps,
                                w1_t[:, kt, ht2 * 128 : (ht2 + 1) * 128],
                                sb_l1T[:, kt, :],
                                start=(kt == 0),
                                stop=(kt == FT - 1),
                            )
                        # relu(x + b1) on DVE: (x add b1) max 0
                        nc.vector.tensor_scalar(
                            sb_hT[:, ht2, :], ps, sb_b1[:, ht2 : ht2 + 1], 0.0,
                            op0=ALU.add, op1=ALU.max,
                        )

                    if MAX_PHASE >= 5:
                        # ============ Phase 5: FFN2 + LN2 + out ============
                        out_r = d_out[:].rearrange("(t p) d -> p t d", p=128)
                        for tt in range(TT):
                            f2pre = ln2p.tile([128, DIM], f32, tag="f2pre")
                            for nh in range(2):
                                ps = ps_f1.tile([128, 384], f32, tag="f2")
                                for kt in range(HT):
                                    nc.tensor.matmul(
                                        ps,
                                        sb_hT[:, kt, tt * 128 : (tt + 1) * 128],
                                        w2_t[:, kt, nh * 384 : (nh + 1) * 384],
                                        start=(kt == 0),
                                        stop=(kt == HT - 1),
                                    )
                                nc.vector.scalar_tensor_tensor(
                                    out=f2pre[:, nh * 384 : (nh + 1) * 384],
                                    in0=ps,
                                    scalar=1.0,
                                    in1=b2_bc[:, nh * 384 : (nh + 1) * 384],
                                    op0=ALU.mult,
                                    op1=ALU.add,
                                )
                            l1b = ln2p.tile([128, DIM], f32, tag="l1b")
                            nc.vector.tensor_add(l1b, sb_l1[:, tt, :], bb2_bc)
                            o_sb = outp.tile([128, DIM], f32, tag="osb")
                            layer_norm_to(o_sb[:], f2pre[:], g2_bc, l1b, ln2p)
                            nc.sync.dma_start(out=out_r[:, tt, :], in_=o_sb)

            if MAX_PHASE >= 3:
                ffnp_cm.__exit__(None, None, None)

    return nc


def _get_nc(finalized=True):
    if "nc" not in _CACHE:
        _CACHE["nc"] = _build_program()
    nc = _CACHE["nc"]
    if finalized and not nc.is_finalized():
        nc.finalize()
    return nc


def make_in_maps(inputs: dict) -> list:
    x = np.asarray(inputs["x_n"], np.float32).reshape(B, S, DIM)
    mask = np.asarray(inputs["mask"]).reshape(B, S)
    w = {
        k: np.ascontiguousarray(np.asarray(inputs[k], np.float32).astype(BF16))
        for k in ("wq", "wk", "wv", "wo", "w1", "w2")
    }
    vecs = {
        "bq": inputs["bq"], "bk": inputs["bk"], "bv": inputs["bv"],
        "bo": inputs["bo"], "b1": inputs["b1"], "b2": inputs["b2"],
        "g1": inputs["ln1_g"], "bb1": inputs["ln1_b"],
        "g2": inputs["ln2_g"], "bb2": inputs["ln2_b"],
    }
    vecs = {k: np.ascontiguousarray(np.asarray(v, np.float32)) for k, v in vecs.items()}
    in_maps = []
    for c in range(N_CORES):
        b, blk = c // NBLK, c % NBLK
        xb = x[b]
        xT = np.ascontiguousarray(xb.T.astype(BF16))
        xblk = np.ascontiguousarray(xb[blk * BLK : (blk + 1) * BLK])
        xTb = np.ascontiguousarray(xblk.T.astype(BF16))
        msc = ((mask[b].astype(np.float32) != 0).astype(np.float32) * ISCALE).astype(
            BF16
        )
        m = {"xT": xT, "xTb": xTb, "xb": xblk, "msc": msc}
        m.update(w)
        m.update(vecs)
        in_maps.append(m)
    return in_maps


def assemble(per_core_out: list) -> np.ndarray:
    blocks = [np.asarray(o, np.float32) for o in per_core_out]
    full = np.concatenate(blocks, axis=0).reshape(B, S, DIM)
    return full


def kernel(**inputs) -> np.ndarray:
    from concourse.bass_utils import run_bass_kernel_spmd

    nc = _get_nc()
    in_maps = make_in_maps(inputs)
    res = run_bass_kernel_spmd(nc, in_maps, list(range(N_CORES)))
    return assemble([r["out"] for r in res.results])


# revision 26
# speedup vs baseline: 1.1817x; 1.1059x over previous
"""Trainium2 Bass kernel for a dense transformer encoder layer.

Model (faithful to the oracle):
  q,k,v = x@wq+bq, x@wk+bk, x@wv+bv          (12 heads, dk=64, DIM=768)
  scores = q@k^T / sqrt(768)  (note: sqrt(dim_model), not sqrt(dk))
  scores[mask==0] = 1e-11  (NOT -inf; masked keys still contribute ~1/Z)
  attn = softmax(scores); z = attn@v; o = z@wo+bo
  l1 = x + LN(o);  ffn = relu(l1@w1+b1)@w2+b2;  out = l1 + LN(ffn)

Sharding: 4096 tokens (B=2,S=2048) split 8 ways -> 512 tokens/core.
Cores 0-3 own batch 0, cores 4-7 batch 1. K/V are computed for the
core's whole batch (redundantly within each 4-core group): measured
faster than all-gathering them (no collective barrier/trigger stalls,
and the extra matmuls keep the PE array HAM-warm).

Masking trick: mask*1/sqrt(768) is folded into K^T at the K-projection
bias-add (scalar_tensor_tensor: (k+bk)*msc), so masked key columns are
exactly 0 -> score 0 -> exp(0)=1.0 == fp32(exp(1e-11)). This makes the
exp scale-free, so it is batched 3 score-tiles per ACTIVATE (amortizes
the ~350-cycle ScalarE instruction overhead).

Softmax denominator comes from a ones column appended to V (attn@v
with M=65); normalization happens after attn@v via a rank-1 matmul
broadcast of 1/sum.
"""

import math
import os
import sys

import numpy as np

for _p in ("/opt/trn_rl_repo", os.path.expanduser("~/.axon_site/_ro/trn_rl_repo")):
    if os.path.isdir(_p) and _p not in sys.path:
        sys.path.insert(0, _p)

import ml_dtypes  # noqa: E402

BF16 = ml_dtypes.bfloat16

DIM = 768
HEADS = 12
DK = 64
HID = 4 * DIM  # 3072
B, S = 2, 2048
N_CORES = 8
BLK = 512            # tokens per core
NBLK = S // BLK      # 4 blocks per batch
EPS = 1e-5
ISCALE = 1.0 / math.sqrt(DIM)

FT = DIM // 128   # 6 feature tiles
TT = BLK // 128   # 4 token tiles per core block
ST = S // 128     # 16 key tiles per batch
HT = HID // 128   # 24 hidden tiles

# exp batching: groups of score k-tiles fused into one ACTIVATE
EXP_GROUPS = [(2 * i, 2 * i + 2) for i in range(8)]
EG = 2  # group width in banks (PSUM: 2*2 sc + 2 z + 2 v = 8 banks)

_CACHE: dict = {}
MAX_PHASE = int(os.environ.get("BASS_KERNEL_PHASES", "5"))


def _build_program():
    import concourse.bass as bass
    import concourse.mybir as mybir
    import concourse.tile as tile
    from concourse import bacc
    from concourse.masks import make_identity

    f32 = mybir.dt.float32
    bf16 = mybir.dt.bfloat16
    AF = mybir.ActivationFunctionType
    ALU = mybir.AluOpType
    AX = mybir.AxisListType

    nc = bacc.Bacc()

    # ---- per-core DRAM I/O ----
    d_xT = nc.dram_tensor("xT", [DIM, S], bf16, kind="ExternalInput")
    d_xTb = nc.dram_tensor("xTb", [DIM, BLK], bf16, kind="ExternalInput")
    d_xb = nc.dram_tensor("xb", [BLK, DIM], f32, kind="ExternalInput")
    d_msc = nc.dram_tensor("msc", [S], bf16, kind="ExternalInput")
    d_wq = nc.dram_tensor("wq", [DIM, DIM], bf16, kind="ExternalInput")
    d_wk = nc.dram_tensor("wk", [DIM, DIM], bf16, kind="ExternalInput")
    d_wv = nc.dram_tensor("wv", [DIM, DIM], bf16, kind="ExternalInput")
    d_wo = nc.dram_tensor("wo", [DIM, DIM], bf16, kind="ExternalInput")
    d_w1 = nc.dram_tensor("w1", [DIM, HID], bf16, kind="ExternalInput")
    d_w2 = nc.dram_tensor("w2", [HID, DIM], bf16, kind="ExternalInput")
    d_bq = nc.dram_tensor("bq", [DIM], f32, kind="ExternalInput")
    d_bk = nc.dram_tensor("bk", [DIM], f32, kind="ExternalInput")
    d_bv = nc.dram_tensor("bv", [DIM], f32, kind="ExternalInput")
    d_bo = nc.dram_tensor("bo", [DIM], f32, kind="ExternalInput")
    d_b1 = nc.dram_tensor("b1", [HID], f32, kind="ExternalInput")
    d_b2 = nc.dram_tensor("b2", [DIM], f32, kind="ExternalInput")
    d_g1 = nc.dram_tensor("g1", [DIM], f32, kind="ExternalInput")
    d_bb1 = nc.dram_tensor("bb1", [DIM], f32, kind="ExternalInput")
    d_g2 = nc.dram_tensor("g2", [DIM], f32, kind="ExternalInput")
    d_bb2 = nc.dram_tensor("bb2", [DIM], f32, kind="ExternalInput")
    d_out = nc.dram_tensor("out", [BLK, DIM], f32, kind="ExternalOutput")

    def bcast_ap(handle, n=128):
        ap = handle[:]
        return bass.AP(tensor=ap.tensor, offset=ap.offset, ap=[[0, n]] + list(ap.ap))

    with tile.TileContext(nc) as tc:
        with (
            tc.tile_pool(name="const", bufs=1) as const,
            tc.tile_pool(name="bigres", bufs=1) as big,
        ):
            # ---------- constants ----------
            # (bq/bk/b1 DMAs are issued inside phase 1, after the weight
            # loads they follow on the sync queue)
            sb_bk = const.tile([128, FT], f32)
            sb_bq = const.tile([128, FT], f32)
            sb_b1 = const.tile([128, HT], f32)
            bo_bc = const.tile([128, DIM], f32)
            nc.gpsimd.dma_start(out=bo_bc, in_=bcast_ap(d_bo))
            b2_bc = const.tile([128, DIM], f32)
            nc.gpsimd.dma_start(out=b2_bc, in_=bcast_ap(d_b2))
            g1_bc = const.tile([128, DIM], f32)
            nc.gpsimd.dma_start(out=g1_bc, in_=bcast_ap(d_g1))
            bb1_bc = const.tile([128, DIM], f32)
            nc.gpsimd.dma_start(out=bb1_bc, in_=bcast_ap(d_bb1))
            g2_bc = const.tile([128, DIM], f32)
            nc.gpsimd.dma_start(out=g2_bc, in_=bcast_ap(d_g2))
            bb2_bc = const.tile([128, DIM], f32)
            nc.gpsimd.dma_start(out=bb2_bc, in_=bcast_ap(d_bb2))
            ident = const.tile([128, 128], f32)
            make_identity(nc, ident[:])
            ones64 = const.tile([1, 64], f32)
            nc.vector.memset(ones64, 1.0)
            eps_t = const.tile([128, 1], f32)
            nc.vector.memset(eps_t, EPS)

            # ---------- persistent activations ----------
            sb_xblk = big.tile([128, TT, DIM], f32)  # residual x (needed ph3)
            sb_l1 = big.tile([128, TT, DIM], f32)
            sb_zT = big.tile([128, FT, BLK], bf16)  # z^T normalized (ph2 -> ph3)

            # attention-scoped residents (freed before phase 3's LN pools)
            attn_res_cm = tc.tile_pool(name="attn_res", bufs=1)
            attn_res = attn_res_cm.__enter__()
            sb_K = attn_res.tile([128, FT, S], bf16)   # K^T*msc, feat-major
            sb_Q = attn_res.tile([128, FT, BLK], bf16)  # Q^T, feat-major
            sb_V = attn_res.tile([128, ST, HEADS, DK + 1], bf16)  # V + ones col

            # ============ Phase 1: Q/K projections ============
            # xw2 (xT, wv, bv) stays live through attention: the V
            # projection is interleaved into the attention rounds to give
            # the PE full-array work while ScalarE runs the exps.
            xw2_cm = tc.tile_pool(name="xw2", bufs=1)
            xw2 = xw2_cm.__enter__()
            sb_xT = xw2.tile([128, FT, S], bf16)
            w_v = xw2.tile([128, FT, DIM], bf16)
            bv_bc = xw2.tile([128, DIM], f32)
            nc.gpsimd.dma_start(out=bv_bc, in_=bcast_ap(d_bv))

            with (
                tc.tile_pool(name="xw", bufs=1) as xw,
                tc.tile_pool(name="ps1", bufs=4, space="PSUM") as ps1,
            ):
                # phase-1-only broadcast constants
                msc_bc = xw.tile([128, S], bf16)
                nc.gpsimd.dma_start(out=msc_bc, in_=bcast_ap(d_msc))
                # critical-path order: Q's inputs (smallest) first, then K's
                # with xT split in halves so K matmuls start early
                w_q = xw.tile([128, FT, DIM], bf16)
                nc.sync.dma_start(
                    out=w_q, in_=d_wq[:].rearrange("(t p) o -> p t o", p=128)
                )
                sb_xTb = xw.tile([128, FT, BLK], bf16)
                nc.sync.dma_start(
                    out=sb_xTb, in_=d_xTb[:].rearrange("(t p) n -> p t n", p=128)
                )
                nc.sync.dma_start(
                    out=sb_bq, in_=d_bq[:].rearrange("(t p) -> p t", p=128)
                )
                w_k = xw.tile([128, FT, DIM], bf16)
                nc.sync.dma_start(
                    out=w_k, in_=d_wk[:].rearrange("(t p) o -> p t o", p=128)
                )
                nc.sync.dma_start(
                    out=sb_xT[:, :, 0 : S // 2],
                    in_=d_xT[:, 0 : S // 2].rearrange("(t p) n -> p t n", p=128),
                )
                nc.sync.dma_start(
                    out=sb_bk, in_=d_bk[:].rearrange("(t p) -> p t", p=128)
                )
                nc.sync.dma_start(
                    out=sb_xT[:, :, S // 2 : S],
                    in_=d_xT[:, S // 2 : S].rearrange("(t p) n -> p t n", p=128),
                )
                nc.sync.dma_start(
                    out=w_v, in_=d_wv[:].rearrange("(t p) o -> p t o", p=128)
                )
                # not needed until later phases: queue behind everything
                nc.sync.dma_start(
                    out=sb_xblk, in_=d_xb[:].rearrange("(t p) d -> p t d", p=128)
                )
                nc.sync.dma_start(
                    out=sb_b1, in_=d_b1[:].rearrange("(t p) -> p t", p=128)
                )

                # Q^T feat-major for the core's block (first DMAs to land)
                for ft in range(FT):
                    ps = ps1.tile([128, 512], f32, tag="p")
                    for kt in range(FT):
                        nc.tensor.matmul(
                            ps,
                            w_q[:, kt, ft * 128 : (ft + 1) * 128],
                            sb_xTb[:, kt, :],
                            start=(kt == 0),
                            stop=(kt == FT - 1),
                        )
                    nc.vector.tensor_scalar_add(
                        sb_Q[:, ft, :], ps, sb_bq[:, ft : ft + 1]
                    )
                # K^T feat-major over the whole batch, (K+bk)*msc fused;
                # nt-outer so the first xT half is enough to start
                for nt in range(S // 512):
                    for ft in range(FT):
                        ps = ps1.tile([128, 512], f32, tag="p")
                        for kt in range(FT):
                            nc.tensor.matmul(
                                ps,
                                w_k[:, kt, ft * 128 : (ft + 1) * 128],
                                sb_xT[:, kt, nt * 512 : (nt + 1) * 512],
                                start=(kt == 0),
                                stop=(kt == FT - 1),
                            )
                        nc.vector.scalar_tensor_tensor(
                            out=sb_K[:, ft, nt * 512 : (nt + 1) * 512],
                            in0=ps,
                            scalar=sb_bk[:, ft : ft + 1],
                            in1=msc_bc[:, nt * 512 : (nt + 1) * 512],
                            op0=ALU.add,
                            op1=ALU.mult,
                        )
                nc.vector.memset(sb_V[:, :, :, DK : DK + 1], 1.0)

            # prefetch wo now -- the DMA overlaps the attention phase
            w_o = big.tile([128, FT, DIM], bf16)
            nc.sync.dma_start(out=w_o, in_=d_wo[:].rearrange("(t p) o -> p t o", p=128))

            if MAX_PHASE >= 2:
                # ============ Phase 2: attention (+ V projection) ============
                # PSUM budget: sc 2x2 + z 2x1 + v 2x1 = 8 banks.
                with (
                    tc.tile_pool(name="expp", bufs=16) as expp,
                    tc.tile_pool(name="attsm", bufs=3) as attsm,
                    tc.tile_pool(name="ps_sc", bufs=2, space="PSUM") as ps_sc,
                    tc.tile_pool(name="ps_z", bufs=2, space="PSUM") as ps_z,
                    tc.tile_pool(name="ps_v", bufs=2, space="PSUM") as ps_v,
                ):
                    def v_chunk(nh, tt):
                        ps = ps_v.tile([128, 384], f32, tag="vp", name="vps")
                        for kt in range(FT):
                            nc.tensor.matmul(
                                ps,
                                sb_xT[:, kt, tt * 128 : (tt + 1) * 128],
                                w_v[:, kt, nh * 384 : (nh + 1) * 384],
                                start=(kt == 0),
                                stop=(kt == FT - 1),
                            )
                        nc.vector.scalar_tensor_tensor(
                            out=sb_V[:, tt, nh * 6 : (nh + 1) * 6, 0:DK],
                            in0=ps[:].rearrange("p (h d) -> p h d", d=DK),
                            scalar=1.0,
                            in1=bv_bc[:, nh * 384 : (nh + 1) * 384].rearrange(
                                "p (h d) -> p h d", d=DK
                            ),
                            op0=ALU.mult,
                            op1=ALU.add,
                        )

                    # V-projection chunks scheduled into rounds: nh0's 16
                    # token-tiles during pair 0 (2/round, just ahead of the
                    # z-matmuls that consume them), nh1's during pairs 1-2.
                    def v_sched(r):
                        if r < 8:
                            return [(0, 2 * r), (0, 2 * r + 1)]
                        if r < 24:
                            return [(1, r - 8)]
                        return []
                    def z_chunk(zps, ets, hp, half, a, b):
                        h = 2 * hp + half
                        for kt2 in range(a, b):
                            nc.tensor.matmul(
                                zps[half][0 : DK + 1, :],
                                sb_V[:, kt2, h, :],
                                ets[half][kt2],
                                start=(kt2 == 0),
                                stop=(kt2 == ST - 1),
                            )

                    def z_tail(zps, ht, half):
                        # 1/Z broadcast lives in partitions 64-127 of the z
                        # tile's own PSUM bank (row 64, the denominator, is
                        # consumed by the reciprocal before being overwritten)
                        ho = half * 64
                        zp = zps[half]
                        rsum = attsm.tile([1, BLK], f32, tag="rsum")
                        nc.vector.reciprocal(rsum, zp[DK : DK + 1, :])
                        nc.tensor.matmul(
                            zp[64:128, :], ones64[:], rsum, start=True, stop=True
                        )
                        rb = attsm.tile([64, BLK], f32, tag="rbs")
                        nc.vector.tensor_copy(rb, zp[64:128, :])
                        nc.vector.tensor_mul(
                            sb_zT[ho : ho + 64, ht, :], zp[0:DK, :], rb
                        )

                    # Flat software pipeline over (pair, exp-group) rounds:
                    # z-matmul chunks run TWO rounds behind their scores so
                    # every PE instruction's inputs (exp tiles) are ready
                    # long before it issues -- a gap-free PE stream lets the
                    # HAM clock-gate reach (and keep) the 2.4 GHz state.
                    NP = HEADS // 2
                    seq = [(hp, gi) for hp in range(NP) for gi in range(len(EXP_GROUPS))]
                    all_ets = [([], []) for _ in range(NP)]
                    all_zps = [None] * NP
                    LAG = 2
                    for idx in range(len(seq) + LAG):
                        for nh, tt in v_sched(idx):
                            v_chunk(nh, tt)
                        if idx < len(seq):
                            hp, gi = seq[idx]
                            ht = hp
                            a, b = EXP_GROUPS[gi]
                            g = b - a
                            if gi == 0:
                                all_zps[hp] = [
                                    ps_z.tile([128, BLK], f32, tag="z", name="zp0"),
                                    ps_z.tile([128, BLK], f32, tag="z", name="zp1"),
                                ]
                            ets = all_ets[hp]
                            for half in (0, 1):
                                ho = half * 64
                                ps = ps_sc.tile(
                                    [128, EG * 512], f32, tag="sc", name="psg"
                                )
                                for j, kt2 in enumerate(range(a, b)):
                                    nc.tensor.matmul(
                                        ps[:, j * 512 : (j + 1) * 512],
                                        sb_K[ho : ho + 64, ht, kt2 * 128 : (kt2 + 1) * 128],
                                        sb_Q[ho : ho + 64, ht, :],
                                        start=True,
                                        stop=True,
                                    )
                                et = expp.tile([128, EG * 512], bf16, tag="exp")
                                nc.scalar.activation(
                                    et[:, : g * 512], ps[:, : g * 512], AF.Exp
                                )
                                for j in range(g):
                                    ets[half].append(et[:, j * 512 : (j + 1) * 512])
                        if idx >= LAG:
                            hp2, gi2 = seq[idx - LAG]
                            pa, pb = EXP_GROUPS[gi2]
                            for half in (0, 1):
                                z_chunk(all_zps[hp2], all_ets[hp2], hp2, half, pa, pb)
                            if gi2 == len(EXP_GROUPS) - 1:
                                for half in (0, 1):
                                    z_tail(all_zps[hp2], hp2, half)

            xw2_cm.__exit__(None, None, None)
            attn_res_cm.__exit__(None, None, None)

            if MAX_PHASE >= 3:
                # ============ Phase 3: O proj + LN1 (+residual) ============
                def layer_norm_to(out_ap, x_ap, g_bc_t, resid_ap, pool):
                    s = pool.tile([128, 1], f32, tag="ln_s")
                    nc.vector.tensor_reduce(s, x_ap, axis=AX.X, op=ALU.add)
                    mean = pool.tile([128, 1], f32, tag="ln_m")
                    nc.scalar.mul(mean, s, 1.0 / DIM)
                    xc = pool.tile([128, DIM], f32, tag="ln_xc")
                    nc.vector.tensor_scalar(xc, x_ap, mean, None, op0=ALU.subtract)
                    junk = pool.tile([128, DIM], f32, tag="ln_j")
                    var = pool.tile([128, 1], f32, tag="ln_v")
                    # (tensor_tensor_reduce crashes the device on this runtime;
                    # scalar_tensor_tensor with accum_out works)
                    nc.vector.scalar_tensor_tensor(
                        out=junk, in0=xc, scalar=1.0, in1=xc,
                        op0=ALU.mult, op1=ALU.mult, accum_out=var,
                    )
                    nc.vector.tensor_scalar_mul(var, var, 1.0 / DIM)
                    sd = pool.tile([128, 1], f32, tag="ln_sd")
                    nc.scalar.activation(sd, var, AF.Sqrt, bias=eps_t[:])
                    rstd = pool.tile([128, 1], f32, tag="ln_r")
                    nc.vector.reciprocal(rstd, sd)
                    t = pool.tile([128, DIM], f32, tag="ln_t")
                    nc.vector.tensor_scalar(t, xc, rstd, None, op0=ALU.mult)
                    tg = pool.tile([128, DIM], f32, tag="ln_tg")
                    nc.vector.tensor_mul(tg, t, g_bc_t)
                    nc.vector.tensor_add(out_ap, tg, resid_ap)

                ffnp_cm = tc.tile_pool(name="ffnp", bufs=1)
                ffnp = ffnp_cm.__enter__()
                # w1 first (feeds FFN1 soon), w2 lands during FFN1; both
                # DMAs overlap phase 3's O-projection + LayerNorm
                w1_t = ffnp.tile([128, FT, HID], bf16)
                nc.sync.dma_start(
                    out=w1_t, in_=d_w1[:].rearrange("(t p) h -> p t h", p=128)
                )
                w2_t = ffnp.tile([128, HT, DIM], bf16)
                nc.sync.dma_start(
                    out=w2_t, in_=d_w2[:].rearrange("(t p) o -> p t o", p=128)
                )
                sb_hT = ffnp.tile([128, HT, BLK], bf16)  # relu(ffn1)^T
                sb_l1T = ffnp.tile([128, FT, BLK], bf16)

                with (
                    tc.tile_pool(name="ln1p", bufs=2) as ln1p,
                    tc.tile_pool(name="ps_o", bufs=4, space="PSUM") as ps_o,
                    tc.tile_pool(name="ps_t1", bufs=2, space="PSUM") as ps_t1,
                ):
                    for tt in range(TT):
                        l1pre = ln1p.tile([128, DIM], f32, tag="l1pre")
                        for nh in range(2):
                            ps = ps_o.tile([128, 384], f32, tag="op")
                            for kt in range(FT):
                                nc.tensor.matmul(
                                    ps,
                                    sb_zT[:, kt, tt * 128 : (tt + 1) * 128],
                                    w_o[:, kt, nh * 384 : (nh + 1) * 384],
                                    start=(kt == 0),
                                    stop=(kt == FT - 1),
                                )
                            nc.vector.scalar_tensor_tensor(
                                out=l1pre[:, nh * 384 : (nh + 1) * 384],
                                in0=ps,
                                scalar=1.0,
                                in1=bo_bc[:, nh * 384 : (nh + 1) * 384],
                                op0=ALU.mult,
                                op1=ALU.add,
                            )
                        xb1 = ln1p.tile([128, DIM], f32, tag="xb1")
                        nc.vector.tensor_add(xb1, sb_xblk[:, tt, :], bb1_bc)
                        layer_norm_to(sb_l1[:, tt, :], l1pre[:], g1_bc, xb1, ln1p)
                        # transpose this l1 tile immediately (feeds FFN1)
                        for ft in range(FT):
                            pst = ps_t1.tile([128, 128], f32, tag="tp")
                            nc.tensor.transpose(
                                pst, sb_l1[:, tt, ft * 128 : (ft + 1) * 128], ident[:]
                            )
                            nc.scalar.copy(
                                sb_l1T[:, ft, tt * 128 : (tt + 1) * 128], pst
                            )

            if MAX_PHASE >= 4:
                # ============ Phase 4+5: FFN (weights already resident) ====
                with (
                    tc.tile_pool(name="ln2p", bufs=2) as ln2p,
                    tc.tile_pool(name="outp", bufs=3) as outp,
                    tc.tile_pool(name="ps_f1", bufs=4, space="PSUM") as ps_f1,
                ):
                    for ht2 in range(HT):
                        ps = ps_f1.tile([128, BLK], f32, tag="f1")
                        for kt in range(FT):
                            nc.tensor.matmul(
                                ps,
                                w1_t[:, kt, ht2 * 128 : (ht2 + 1) * 128],
                                sb_l1T[:, kt, :],
                                start=(kt == 0),
                                stop=(kt == FT - 1),
                            )
                        # relu(x + b1) on DVE: (x add b1) max 0
                        nc.vector.tensor_scalar(
                            sb_hT[:, ht2, :], ps, sb_b1[:, ht2 : ht2 + 1], 0.0,
                            op0=ALU.add, op1=ALU.max,
                        )

                    if MAX_PHASE >= 5:
                        # ============ Phase 5: FFN2 + LN2 + out ============
                        out_r = d_out[:].rearrange("(t p) d -> p t d", p=128)
                        for tt in range(TT):
                            f2pre = ln2p.tile([128, DIM], f32, tag="f2pre")
                            for nh in range(2):
                                ps = ps_f1.tile([128, 384], f32, tag="f2")
                                for kt in range(HT):
                                    nc.tensor.matmul(
                                        ps,
                                        sb_hT[:, kt, tt * 128 : (tt + 1) * 128],
                                        w2_t[:, kt, nh * 384 : (nh + 1) * 384],
                                        start=(kt == 0),
                                        stop=(kt == HT - 1),
                                    )
                                nc.vector.scalar_tensor_tensor(
                                    out=f2pre[:, nh * 384 : (nh + 1) * 384],
                                    in0=ps,
                                    scalar=1.0,
                                    in1=b2_bc[:, nh * 384 : (nh + 1) * 384],
                                    op0=ALU.mult,
                                    op1=ALU.add,
                                )
                            l1b = ln2p.tile([128, DIM], f32, tag="l1b")
                            nc.vector.tensor_add(l1b, sb_l1[:, tt, :], bb2_bc)
                            o_sb = outp.tile([128, DIM], f32, tag="osb")
                            layer_norm_to(o_sb[:], f2pre[:], g2_bc, l1b, ln2p)
                            nc.sync.dma_start(out=out_r[:, tt, :], in_=o_sb)

            if MAX_PHASE >= 3:
                ffnp_cm.__exit__(None, None, None)

    return nc


def _get_nc(finalized=True):
    if "nc" not in _CACHE:
        _CACHE["nc"] = _build_program()
    nc = _CACHE["nc"]
    if finalized and not nc.is_finalized():
        nc.finalize()
    return nc


def make_in_maps(inputs: dict) -> list:
    x = np.asarray(inputs["x_n"], np.float32).reshape(B, S, DIM)
    mask = np.asarray(inputs["mask"]).reshape(B, S)
    w = {
        k: np.ascontiguousarray(np.asarray(inputs[k], np.float32).astype(BF16))
        for k in ("wq", "wk", "wv", "wo", "w1", "w2")
    }
    vecs = {
        "bq": inputs["bq"], "bk": inputs["bk"], "bv": inputs["bv"],
        "bo": inputs["bo"], "b1": inputs["b1"], "b2": inputs["b2"],
        "g1": inputs["ln1_g"], "bb1": inputs["ln1_b"],
        "g2": inputs["ln2_g"], "bb2": inputs["ln2_b"],
    }
    vecs = {k: np.ascontiguousarray(np.asarray(v, np.float32)) for k, v in vecs.items()}
    in_maps = []
    for c in range(N_CORES):
        b, blk = c // NBLK, c % NBLK
        xb = x[b]
        xT = np.ascontiguousarray(xb.T.astype(BF16))
        xblk = np.ascontiguousarray(xb[blk * BLK : (blk + 1) * BLK])
        xTb = np.ascontiguousarray(xblk.T.astype(BF16))
        msc = ((mask[b].astype(np.float32) != 0).astype(np.float32) * ISCALE).astype(
            BF16
        )
        m = {"xT": xT, "xTb": xTb, "xb": xblk, "msc": msc}
        m.update(w)
        m.update(vecs)
        in_maps.append(m)
    return in_maps


def assemble(per_core_out: list) -> np.ndarray:
    blocks = [np.asarray(o, np.float32) for o in per_core_out]
    full = np.concatenate(blocks, axis=0).reshape(B, S, DIM)
    return full


def kernel(**inputs) -> np.ndarray:
    from concourse.bass_utils import run_bass_kernel_spmd

    nc = _get_nc()
    in_maps = make_in_maps(inputs)
    res = run_bass_kernel_spmd(nc, in_maps, list(range(N_CORES)))
    return assemble([r["out"] for r in res.results])


# revision 37
# speedup vs baseline: 1.2258x; 1.0373x over previous
"""Trainium2 Bass kernel for a dense transformer encoder layer.

Model (faithful to the oracle):
  q,k,v = x@wq+bq, x@wk+bk, x@wv+bv          (12 heads, dk=64, DIM=768)
  scores = q@k^T / sqrt(768)  (note: sqrt(dim_model), not sqrt(dk))
  scores[mask==0] = 1e-11  (NOT -inf; masked keys still contribute ~1/Z)
  attn = softmax(scores); z = attn@v; o = z@wo+bo
  l1 = x + LN(o);  ffn = relu(l1@w1+b1)@w2+b2;  out = l1 + LN(ffn)

Sharding: 4096 tokens (B=2,S=2048) split 8 ways -> 512 tokens/core.
Cores 0-3 own batch 0, cores 4-7 batch 1. K/V are computed for the
core's whole batch (redundantly within each 4-core group): measured
faster than all-gathering them (no collective barrier/trigger stalls,
and the extra matmuls keep the PE array HAM-warm).

Masking trick: mask*1/sqrt(768) is folded into K^T at the K-projection
bias-add (scalar_tensor_tensor: (k+bk)*msc), so masked key columns are
exactly 0 -> score 0 -> exp(0)=1.0 == fp32(exp(1e-11)). This makes the
exp scale-free, so it is batched 3 score-tiles per ACTIVATE (amortizes
the ~350-cycle ScalarE instruction overhead).

Softmax denominator comes from a ones column appended to V (attn@v
with M=65); normalization happens after attn@v via a rank-1 matmul
broadcast of 1/sum.
"""

import math
import os
import sys

import numpy as np

for _p in ("/opt/trn_rl_repo", os.path.expanduser("~/.axon_site/_ro/trn_rl_repo")):
    if os.path.isdir(_p) and _p not in sys.path:
        sys.path.insert(0, _p)

import ml_dtypes  # noqa: E402

BF16 = ml_dtypes.bfloat16

DIM = 768
HEADS = 12
DK = 64
HID = 4 * DIM  # 3072
B, S = 2, 2048
N_CORES = 8
BLK = 512            # tokens per core
NBLK = S // BLK      # 4 blocks per batch
EPS = 1e-5
ISCALE = 1.0 / math.sqrt(DIM)

FT = DIM // 128   # 6 feature tiles
TT = BLK // 128   # 4 token tiles per core block
ST = S // 128     # 16 key tiles per batch
HT = HID // 128   # 24 hidden tiles

# exp batching: groups of score k-tiles fused into one ACTIVATE
EXP_GROUPS = [(2 * i, 2 * i + 2) for i in range(8)]
EG = 2  # group width in banks (PSUM: 2*2 sc + 2 z + 2 v = 8 banks)

_CACHE: dict = {}
MAX_PHASE = int(os.environ.get("BASS_KERNEL_PHASES", "5"))


def _build_program():
    import concourse.bass as bass
    import concourse.mybir as mybir
    import concourse.tile as tile
    from concourse import bacc
    from concourse.masks import make_identity

    f32 = mybir.dt.float32
    bf16 = mybir.dt.bfloat16
    AF = mybir.ActivationFunctionType
    ALU = mybir.AluOpType
    AX = mybir.AxisListType

    nc = bacc.Bacc()

    # ---- per-core DRAM I/O ----
    d_xT = nc.dram_tensor("xT", [DIM, S], bf16, kind="ExternalInput")
    d_xTb = nc.dram_tensor("xTb", [DIM, BLK], bf16, kind="ExternalInput")
    d_xb = nc.dram_tensor("xb", [BLK, DIM], f32, kind="ExternalInput")
    d_msc = nc.dram_tensor("msc", [S], bf16, kind="ExternalInput")
    d_wq = nc.dram_tensor("wq", [DIM, DIM], bf16, kind="ExternalInput")
    d_wk = nc.dram_tensor("wk", [DIM, DIM], bf16, kind="ExternalInput")
    d_wv = nc.dram_tensor("wv", [DIM, DIM], bf16, kind="ExternalInput")
    d_wo = nc.dram_tensor("wo", [DIM, DIM], bf16, kind="ExternalInput")
    d_w1 = nc.dram_tensor("w1", [DIM, HID], bf16, kind="ExternalInput")
    d_w2 = nc.dram_tensor("w2", [HID, DIM], bf16, kind="ExternalInput")
    d_bq = nc.dram_tensor("bq", [DIM], f32, kind="ExternalInput")
    d_bk = nc.dram_tensor("bk", [DIM], f32, kind="ExternalInput")
    d_bv = nc.dram_tensor("bv", [DIM], f32, kind="ExternalInput")
    d_bo = nc.dram_tensor("bo", [DIM], f32, kind="ExternalInput")
    d_b1 = nc.dram_tensor("b1", [HID], f32, kind="ExternalInput")
    d_b2 = nc.dram_tensor("b2", [DIM], f32, kind="ExternalInput")
    d_g1 = nc.dram_tensor("g1", [DIM], f32, kind="ExternalInput")
    d_bb1 = nc.dram_tensor("bb1", [DIM], f32, kind="ExternalInput")
    d_g2 = nc.dram_tensor("g2", [DIM], f32, kind="ExternalInput")
    d_bb2 = nc.dram_tensor("bb2", [DIM], f32, kind="ExternalInput")
    d_out = nc.dram_tensor("out", [BLK, DIM], f32, kind="ExternalOutput")

    def bcast_ap(handle, n=128):
        ap = handle[:]
        return bass.AP(tensor=ap.tensor, offset=ap.offset, ap=[[0, n]] + list(ap.ap))

    with tile.TileContext(nc) as tc:
        with (
            tc.tile_pool(name="const", bufs=1) as const,
            tc.tile_pool(name="bigres", bufs=1) as big,
        ):
            # ---------- constants ----------
            # (bq/bk/b1 DMAs are issued inside phase 1, after the weight
            # loads they follow on the sync queue)
            sb_bk = const.tile([128, FT], f32)
            sb_bq = const.tile([128, FT], f32)
            sb_b1 = const.tile([128, HT], f32)
            bo_bc = const.tile([128, DIM], f32)
            nc.gpsimd.dma_start(out=bo_bc, in_=bcast_ap(d_bo))
            b2_bc = const.tile([128, DIM], f32)
            nc.gpsimd.dma_start(out=b2_bc, in_=bcast_ap(d_b2))
            g1_bc = const.tile([128, DIM], f32)
            nc.gpsimd.dma_start(out=g1_bc, in_=bcast_ap(d_g1))
            bb1_bc = const.tile([128, DIM], f32)
            nc.gpsimd.dma_start(out=bb1_bc, in_=bcast_ap(d_bb1))
            g2_bc = const.tile([128, DIM], f32)
            nc.gpsimd.dma_start(out=g2_bc, in_=bcast_ap(d_g2))
            bb2_bc = const.tile([128, DIM], f32)
            nc.gpsimd.dma_start(out=bb2_bc, in_=bcast_ap(d_bb2))
            ident = const.tile([128, 128], f32)
            make_identity(nc, ident[:])
            ones64 = const.tile([1, 64], f32)
            nc.vector.memset(ones64, 1.0)
            eps_t = const.tile([128, 1], f32)
            nc.vector.memset(eps_t, EPS)

            # ---------- persistent activations ----------
            sb_xblk = big.tile([128, TT, DIM], f32)  # residual x (needed ph3)
            sb_l1 = big.tile([128, TT, DIM], f32)
            sb_zT = big.tile([128, FT, BLK], bf16)  # z^T normalized (ph2 -> ph3)

            # O-projection partial accumulator: head-pairs 0-2's contribution
            # (+bo) is accumulated during the attention tail rounds
            opre_cm = tc.tile_pool(name="opre", bufs=1)
            opre = opre_cm.__enter__()
            o_acc = opre.tile([128, TT, DIM], f32)

            # attention-scoped residents (freed before phase 3's LN pools)
            attn_res_cm = tc.tile_pool(name="attn_res", bufs=1)
            attn_res = attn_res_cm.__enter__()
            sb_K = attn_res.tile([128, FT, S], bf16)   # K^T*msc, feat-major
            sb_Q = attn_res.tile([128, FT, BLK], bf16)  # Q^T, feat-major
            sb_V = attn_res.tile([128, ST, HEADS, DK + 1], bf16)  # V + ones col

            # ============ Phase 1: Q/K projections ============
            # xw2 (xT, wv, bv) stays live through attention: the V
            # projection is interleaved into the attention rounds to give
            # the PE full-array work while ScalarE runs the exps.
            xw2_cm = tc.tile_pool(name="xw2", bufs=1)
            xw2 = xw2_cm.__enter__()
            sb_xT = xw2.tile([128, FT, S], bf16)
            w_v = xw2.tile([128, FT, DIM], bf16)
            bv_bc = xw2.tile([128, DIM], f32)
            nc.gpsimd.dma_start(out=bv_bc, in_=bcast_ap(d_bv))

            with (
                tc.tile_pool(name="xw", bufs=1) as xw,
                tc.tile_pool(name="ps1", bufs=4, space="PSUM") as ps1,
            ):
                # phase-1-only broadcast constants
                msc_bc = xw.tile([128, S], bf16)
                nc.gpsimd.dma_start(out=msc_bc, in_=bcast_ap(d_msc))
                # critical-path order: Q's inputs (smallest) first, then K's
                # with xT split in halves so K matmuls start early
                w_q = xw.tile([128, FT, DIM], bf16)
                nc.sync.dma_start(
                    out=w_q, in_=d_wq[:].rearrange("(t p) o -> p t o", p=128)
                )
                sb_xTb = xw.tile([128, FT, BLK], bf16)
                nc.sync.dma_start(
                    out=sb_xTb, in_=d_xTb[:].rearrange("(t p) n -> p t n", p=128)
                )
                nc.sync.dma_start(
                    out=sb_bq, in_=d_bq[:].rearrange("(t p) -> p t", p=128)
                )
                # w_k rides the gpsimd (SWDGE) queue, in parallel with the
                # sync-queue x loads
                w_k = xw.tile([128, FT, DIM], bf16)
                nc.gpsimd.dma_start(
                    out=w_k, in_=d_wk[:].rearrange("(t p) o -> p t o", p=128)
                )
                nc.sync.dma_start(
                    out=sb_xT[:, :, 0 : S // 2],
                    in_=d_xT[:, 0 : S // 2].rearrange("(t p) n -> p t n", p=128),
                )
                nc.sync.dma_start(
                    out=sb_bk, in_=d_bk[:].rearrange("(t p) -> p t", p=128)
                )
                nc.sync.dma_start(
                    out=sb_xT[:, :, S // 2 : S],
                    in_=d_xT[:, S // 2 : S].rearrange("(t p) n -> p t n", p=128),
                )
                nc.sync.dma_start(
                    out=w_v, in_=d_wv[:].rearrange("(t p) o -> p t o", p=128)
                )
                # not needed until later phases: queue behind everything
                nc.sync.dma_start(
                    out=sb_xblk, in_=d_xb[:].rearrange("(t p) d -> p t d", p=128)
                )
                nc.sync.dma_start(
                    out=sb_b1, in_=d_b1[:].rearrange("(t p) -> p t", p=128)
                )

                # Q^T feat-major for the core's block (first DMAs to land)
                for ft in range(FT):
                    ps = ps1.tile([128, 512], f32, tag="p")
                    for kt in range(FT):
                        nc.tensor.matmul(
                            ps,
                            w_q[:, kt, ft * 128 : (ft + 1) * 128],
                            sb_xTb[:, kt, :],
                            start=(kt == 0),
                            stop=(kt == FT - 1),
                        )
                    nc.vector.tensor_scalar_add(
                        sb_Q[:, ft, :], ps, sb_bq[:, ft : ft + 1]
                    )
                # K^T feat-major over the whole batch, (K+bk)*msc fused;
                # nt-outer so the first xT half is enough to start
                for nt in range(S // 512):
                    for ft in range(FT):
                        ps = ps1.tile([128, 512], f32, tag="p")
                        for kt in range(FT):
                            nc.tensor.matmul(
                                ps,
                                w_k[:, kt, ft * 128 : (ft + 1) * 128],
                                sb_xT[:, kt, nt * 512 : (nt + 1) * 512],
                                start=(kt == 0),
                                stop=(kt == FT - 1),
                            )
                        nc.vector.scalar_tensor_tensor(
                            out=sb_K[:, ft, nt * 512 : (nt + 1) * 512],
                            in0=ps,
                            scalar=sb_bk[:, ft : ft + 1],
                            in1=msc_bc[:, nt * 512 : (nt + 1) * 512],
                            op0=ALU.add,
                            op1=ALU.mult,
                        )
                nc.vector.memset(sb_V[:, :, :, DK : DK + 1], 1.0)

            # prefetch wo now -- the DMA overlaps the attention phase
            w_o = big.tile([128, FT, DIM], bf16)
            nc.sync.dma_start(out=w_o, in_=d_wo[:].rearrange("(t p) o -> p t o", p=128))

            if MAX_PHASE >= 2:
                # ============ Phase 2: attention (+ V projection) ============
                # PSUM budget: sc 2x2 + z 2x1 + v 2x1 = 8 banks.
                with (
                    tc.tile_pool(name="expp", bufs=12) as expp,
                    tc.tile_pool(name="attsm", bufs=3) as attsm,
                    tc.tile_pool(name="ps_sc", bufs=2, space="PSUM") as ps_sc,
                    tc.tile_pool(name="ps_z", bufs=2, space="PSUM") as ps_z,
                    tc.tile_pool(name="ps_v", bufs=2, space="PSUM") as ps_v,
                ):
                    def v_chunk(nh, tt):
                        ps = ps_v.tile([128, 384], f32, tag="vp", name="vps")
                        for kt in range(FT):
                            nc.tensor.matmul(
                                ps,
                                sb_xT[:, kt, tt * 128 : (tt + 1) * 128],
                                w_v[:, kt, nh * 384 : (nh + 1) * 384],
                                start=(kt == 0),
                                stop=(kt == FT - 1),
                            )
                        nc.vector.scalar_tensor_tensor(
                            out=sb_V[:, tt, nh * 6 : (nh + 1) * 6, 0:DK],
                            in0=ps[:].rearrange("p (h d) -> p h d", d=DK),
                            scalar=1.0,
                            in1=bv_bc[:, nh * 384 : (nh + 1) * 384].rearrange(
                                "p (h d) -> p h d", d=DK
                            ),
                            op0=ALU.mult,
                            op1=ALU.add,
                        )

                    # V-projection chunks scheduled into rounds: nh0's 16
                    # token-tiles during pair 0 (2/round, just ahead of the
                    # z-matmuls that consume them), nh1's during pairs 1-2.
                    def v_sched(r):
                        if r < 8:
                            return [(0, 2 * r), (0, 2 * r + 1)]
                        if r < 24:
                            return [(1, r - 8)]
                        return []

                    def o_chunk(tt, nh):
                        # accumulate head-pairs 0-2's O-projection (+bo);
                        # zT for those pairs is final by round 26
                        ps = ps_v.tile([128, 384], f32, tag="vp", name="ops")
                        for kt in range(3):
                            nc.tensor.matmul(
                                ps,
                                sb_zT[:, kt, tt * 128 : (tt + 1) * 128],
                                w_o[:, kt, nh * 384 : (nh + 1) * 384],
                                start=(kt == 0),
                                stop=(kt == 2),
                            )
                        nc.vector.scalar_tensor_tensor(
                            out=o_acc[:, tt, nh * 384 : (nh + 1) * 384],
                            in0=ps,
                            scalar=1.0,
                            in1=bo_bc[:, nh * 384 : (nh + 1) * 384],
                            op0=ALU.mult,
                            op1=ALU.add,
                        )

                    O_TARGETS = [(tt, nh) for tt in range(TT) for nh in range(2)]
                    o_sched = {26 + (11 * k) // 4: k for k in range(8)}
                    def z_chunk(zps, ets, hp, half, a, b):
                        h = 2 * hp + half
                        for kt2 in range(a, b):
                            nc.tensor.matmul(
                                zps[half][0 : DK + 1, :],
                                sb_V[:, kt2, h, :],
                                ets[half][kt2],
                                start=(kt2 == 0),
                                stop=(kt2 == ST - 1),
                            )

                    def z_tail(zps, ht, half):
                        # 1/Z broadcast lives in partitions 64-127 of the z
                        # tile's own PSUM bank (row 64, the denominator, is
                        # consumed by the reciprocal before being overwritten)
                        ho = half * 64
                        zp = zps[half]
                        rsum = attsm.tile([1, BLK], f32, tag="rsum")
                        nc.vector.reciprocal(rsum, zp[DK : DK + 1, :])
                        nc.tensor.matmul(
                            zp[64:128, :], ones64[:], rsum, start=True, stop=True
                        )
                        rb = attsm.tile([64, BLK], f32, tag="rbs")
                        nc.vector.tensor_copy(rb, zp[64:128, :])
                        nc.vector.tensor_mul(
                            sb_zT[ho : ho + 64, ht, :], zp[0:DK, :], rb
                        )

                    # Flat software pipeline over (pair, exp-group) rounds:
                    # z-matmul chunks run TWO rounds behind their scores so
                    # every PE instruction's inputs (exp tiles) are ready
                    # long before it issues -- a gap-free PE stream lets the
                    # HAM clock-gate reach (and keep) the 2.4 GHz state.
                    NP = HEADS // 2
                    seq = [(hp, gi) for hp in range(NP) for gi in range(len(EXP_GROUPS))]
                    all_ets = [([], []) for _ in range(NP)]
                    all_zps = [None] * NP
                    LAG = 2
                    for idx in range(len(seq) + LAG):
                        for nh, tt in v_sched(idx):
                            v_chunk(nh, tt)
                        if idx in o_sched:
                            tt_o, nh_o = O_TARGETS[o_sched[idx]]
                            o_chunk(tt_o, nh_o)
                        if idx < len(seq):
                            hp, gi = seq[idx]
                            ht = hp
                            a, b = EXP_GROUPS[gi]
                            g = b - a
                            if gi == 0:
                                all_zps[hp] = [
                                    ps_z.tile([128, BLK], f32, tag="z", name="zp0"),
                                    ps_z.tile([128, BLK], f32, tag="z", name="zp1"),
                                ]
                            ets = all_ets[hp]
                            for half in (0, 1):
                                ho = half * 64
                                ps = ps_sc.tile(
                                    [128, EG * 512], f32, tag="sc", name="psg"
                                )
                                for j, kt2 in enumerate(range(a, b)):
                                    nc.tensor.matmul(
                                        ps[:, j * 512 : (j + 1) * 512],
                                        sb_K[ho : ho + 64, ht, kt2 * 128 : (kt2 + 1) * 128],
                                        sb_Q[ho : ho + 64, ht, :],
                                        start=True,
                                        stop=True,
                                    )
                                et = expp.tile([128, EG * 512], bf16, tag="exp")
                                nc.scalar.activation(
                                    et[:, : g * 512], ps[:, : g * 512], AF.Exp
                                )
                                for j in range(g):
                                    ets[half].append(et[:, j * 512 : (j + 1) * 512])
                        if idx >= LAG:
                            hp2, gi2 = seq[idx - LAG]
                            pa, pb = EXP_GROUPS[gi2]
                            for half in (0, 1):
                                z_chunk(all_zps[hp2], all_ets[hp2], hp2, half, pa, pb)
                            if gi2 == len(EXP_GROUPS) - 1:
                                for half in (0, 1):
                                    z_tail(all_zps[hp2], hp2, half)

            xw2_cm.__exit__(None, None, None)
            attn_res_cm.__exit__(None, None, None)

            if MAX_PHASE >= 3:
                # ============ Phase 3: O proj + LN1 (+residual) ============
                def layer_norm_to(out_ap, x_ap, g_bc_t, resid_ap, pool):
                    s = pool.tile([128, 1], f32, tag="ln_s")
                    nc.vector.tensor_reduce(s, x_ap, axis=AX.X, op=ALU.add)
                    mean = pool.tile([128, 1], f32, tag="ln_m")
                    nc.scalar.mul(mean, s, 1.0 / DIM)
                    xc = pool.tile([128, DIM], f32, tag="ln_xc")
                    nc.vector.tensor_scalar(xc, x_ap, mean, None, op0=ALU.subtract)
                    junk = pool.tile([128, DIM], bf16, tag="ln_j")
                    var = pool.tile([128, 1], f32, tag="ln_v")
                    # (tensor_tensor_reduce crashes the device on this runtime;
                    # scalar_tensor_tensor with accum_out works)
                    nc.vector.scalar_tensor_tensor(
                        out=junk, in0=xc, scalar=1.0, in1=xc,
                        op0=ALU.mult, op1=ALU.mult, accum_out=var,
                    )
                    nc.vector.tensor_scalar_mul(var, var, 1.0 / DIM)
                    sd = pool.tile([128, 1], f32, tag="ln_sd")
                    nc.scalar.activation(sd, var, AF.Sqrt, bias=eps_t[:])
                    rstd = pool.tile([128, 1], f32, tag="ln_r")
                    nc.vector.reciprocal(rstd, sd)
                    tg = pool.tile([128, DIM], f32, tag="ln_tg")
                    nc.vector.scalar_tensor_tensor(
                        out=tg, in0=xc, scalar=rstd, in1=g_bc_t,
                        op0=ALU.mult, op1=ALU.mult,
                    )
                    nc.vector.tensor_add(out_ap, tg, resid_ap)

                ffnp_cm = tc.tile_pool(name="ffnp", bufs=1)
                ffnp = ffnp_cm.__enter__()
                # w1 first (feeds FFN1 soon; split so the first FFN1 matmuls
                # start after half the load), w2 lands during FFN1; the DMAs
                # overlap phase 3's O-projection + LayerNorm
                w1_t = ffnp.tile([128, FT, HID], bf16)
                nc.sync.dma_start(
                    out=w1_t[:, :, 0 : HID // 2],
                    in_=d_w1[:, 0 : HID // 2].rearrange("(t p) h -> p t h", p=128),
                )
                nc.sync.dma_start(
                    out=w1_t[:, :, HID // 2 : HID],
                    in_=d_w1[:, HID // 2 : HID].rearrange("(t p) h -> p t h", p=128),
                )
                w2_t = ffnp.tile([128, HT, DIM], bf16)
                nc.sync.dma_start(
                    out=w2_t, in_=d_w2[:].rearrange("(t p) o -> p t o", p=128)
                )
                sb_hT = ffnp.tile([128, HT, BLK], bf16)  # relu(ffn1)^T
                sb_l1T = ffnp.tile([128, FT, BLK], bf16)

                with (
                    tc.tile_pool(name="ln1p", bufs=2) as ln1p,
                    tc.tile_pool(name="ps_o", bufs=4, space="PSUM") as ps_o,
                    tc.tile_pool(name="ps_t1", bufs=2, space="PSUM") as ps_t1,
                ):
                    for tt in range(TT):
                        l1pre = ln1p.tile([128, DIM], f32, tag="l1pre")
                        for nh in range(2):
                            ps = ps_o.tile([128, 384], f32, tag="op")
                            for kt in range(3, FT):
                                nc.tensor.matmul(
                                    ps,
                                    sb_zT[:, kt, tt * 128 : (tt + 1) * 128],
                                    w_o[:, kt, nh * 384 : (nh + 1) * 384],
                                    start=(kt == 3),
                                    stop=(kt == FT - 1),
                                )
                            # pairs 0-2 (+bo) were accumulated into o_acc
                            # during the attention tail
                            nc.vector.scalar_tensor_tensor(
                                out=l1pre[:, nh * 384 : (nh + 1) * 384],
                                in0=ps,
                                scalar=1.0,
                                in1=o_acc[:, tt, nh * 384 : (nh + 1) * 384],
                                op0=ALU.mult,
                                op1=ALU.add,
                            )
                        xb1 = ln1p.tile([128, DIM], f32, tag="xb1")
                        nc.vector.tensor_add(xb1, sb_xblk[:, tt, :], bb1_bc)
                        layer_norm_to(sb_l1[:, tt, :], l1pre[:], g1_bc, xb1, ln1p)
                        # transpose this l1 tile immediately (feeds FFN1)
                        for ft in range(FT):
                            pst = ps_t1.tile([128, 128], f32, tag="tp")
                            nc.tensor.transpose(
                                pst, sb_l1[:, tt, ft * 128 : (ft + 1) * 128], ident[:]
                            )
                            nc.scalar.copy(
                                sb_l1T[:, ft, tt * 128 : (tt + 1) * 128], pst
                            )

            if MAX_PHASE >= 4:
                # ============ Phase 4+5: FFN (weights already resident) ====
                with (
                    tc.tile_pool(name="ln2p", bufs=2) as ln2p,
                    tc.tile_pool(name="outp", bufs=2) as outp,
                    tc.tile_pool(name="ps_f1", bufs=4, space="PSUM") as ps_f1,
                ):
                    for ht2 in range(HT):
                        ps = ps_f1.tile([128, BLK], f32, tag="f1")
                        for kt in range(FT):
                            nc.tensor.matmul(
                                ps,
                                w1_t[:, kt, ht2 * 128 : (ht2 + 1) * 128],
                                sb_l1T[:, kt, :],
                                start=(kt == 0),
                                stop=(kt == FT - 1),
                            )
                        # relu(x + b1) on DVE: (x add b1) max 0
                        nc.vector.tensor_scalar(
                            sb_hT[:, ht2, :], ps, sb_b1[:, ht2 : ht2 + 1], 0.0,
                            op0=ALU.add, op1=ALU.max,
                        )

                    if MAX_PHASE >= 5:
                        # ============ Phase 5: FFN2 + LN2 + out ============
                        out_r = d_out[:].rearrange("(t p) d -> p t d", p=128)
                        for tt in range(TT):
                            f2pre = ln2p.tile([128, DIM], f32, tag="f2pre")
                            for nh in range(2):
                                ps = ps_f1.tile([128, 384], f32, tag="f2")
                                for kt in range(HT):
                                    nc.tensor.matmul(
                                        ps,
                                        sb_hT[:, kt, tt * 128 : (tt + 1) * 128],
                                        w2_t[:, kt, nh * 384 : (nh + 1) * 384],
                                        start=(kt == 0),
                                        stop=(kt == HT - 1),
                                    )
                                nc.vector.scalar_tensor_tensor(
                                    out=f2pre[:, nh * 384 : (nh + 1) * 384],
                                    in0=ps,
                                    scalar=1.0,
                                    in1=b2_bc[:, nh * 384 : (nh + 1) * 384],
                                    op0=ALU.mult,
                                    op1=ALU.add,
                                )
                            l1b = ln2p.tile([128, DIM], f32, tag="l1b")
                            nc.vector.tensor_add(l1b, sb_l1[:, tt, :], bb2_bc)
                            o_sb = outp.tile([128, DIM], f32, tag="osb")
                            layer_norm_to(o_sb[:], f2pre[:], g2_bc, l1b, ln2p)
                            nc.sync.dma_start(out=out_r[:, tt, :], in_=o_sb)

            if MAX_PHASE >= 3:
                ffnp_cm.__exit__(None, None, None)
            opre_cm.__exit__(None, None, None)

    return nc


def _get_nc(finalized=True):
    if "nc" not in _CACHE:
        _CACHE["nc"] = _build_program()
    nc = _CACHE["nc"]
    if finalized and not nc.is_finalized():
        nc.finalize()
    return nc


def make_in_maps(inputs: dict) -> list:
    x = np.asarray(inputs["x_n"], np.float32).reshape(B, S, DIM)
    mask = np.asarray(inputs["mask"]).reshape(B, S)
    w = {
        k: np.ascontiguousarray(np.asarray(inputs[k], np.float32).astype(BF16))
        for k in ("wq", "wk", "wv", "wo", "w1", "w2")
    }
    vecs = {
        "bq": inputs["bq"], "bk": inputs["bk"], "bv": inputs["bv"],
        "bo": inputs["bo"], "b1": inputs["b1"], "b2": inputs["b2"],
        "g1": inputs["ln1_g"], "bb1": inputs["ln1_b"],
        "g2": inputs["ln2_g"], "bb2": inputs["ln2_b"],
    }
    vecs = {k: np.ascontiguousarray(np.asarray(v, np.float32)) for k, v in vecs.items()}
    in_maps = []
    for c in range(N_CORES):
        b, blk = c // NBLK, c % NBLK
        xb = x[b]
        xT = np.ascontiguousarray(xb.T.astype(BF16))
        xblk = np.ascontiguousarray(xb[blk * BLK : (blk + 1) * BLK])
        xTb = np.ascontiguousarray(xblk.T.astype(BF16))
        msc = ((mask[b].astype(np.float32) != 0).astype(np.float32) * ISCALE).astype(
            BF16
        )
        m = {"xT": xT, "xTb": xTb, "xb": xblk, "msc": msc}
        m.update(w)
        m.update(vecs)
        in_maps.append(m)
    return in_maps


def assemble(per_core_out: list) -> np.ndarray:
    blocks = [np.asarray(o, np.float32) for o in per_core_out]
    full = np.concatenate(blocks, axis=0).reshape(B, S, DIM)
    return full


def kernel(**inputs) -> np.ndarray:
    from concourse.bass_utils import run_bass_kernel_spmd

    nc = _get_nc()
    in_maps = make_in_maps(inputs)
    res = run_bass_kernel_spmd(nc, in_maps, list(range(N_CORES)))
    return assemble([r["out"] for r in res.results])
